# revision 65
# baseline (speedup 1.0000x reference)
"""Trainium2 Bass kernel for causal multi-head attention with NeoX RoPE.

Problem: x[2, 2048, 1024], 16 heads x d_head 64, rotary over all 64 dims,
causal softmax, output projection.

Sharding: every core holds 2 heads ({2c, 2c+1}) and processes BOTH
batches.  After a per-q-chunk 8-core AllToAll of the normalized z shards,
each core contracts all 16 heads locally and writes its own [128 x 1024]
output rows per chunk.

v3 scheduling: all work is decomposed into ~0.5-1.1us units (proj kt
halves, V-proj per s-tile, AV 2-ktile groups, outproj halves, rope,
normalize, DMA) drained between score tiles with a cost budget, so the
PE never idles while exp paces the score psum recycle.  AV of a stage
runs inside the SAME stage's burst (gated on exp progress), which keeps
the tail short.  Collective-chain DMAs (z_send/zall/out) live on the
Sync queue; GpSimd only does affine_select + partition_broadcast, so the
collective wait never head-of-line-blocks the causal mask.  The final
chunk's outproj(2) is reserved to overlap the last AllToAll.
"""

import numpy as np
import ml_dtypes

S = 2048
D = 1024
NH = 16
DH = 64
SCALE = 8.0
ROT_BASE = 10000.0
N_CORES = 8
QCHUNK = 512     # q chunk (free dim of score matmuls)
NCHUNK = S // QCHUNK
KTILE = 128
BF = ml_dtypes.bfloat16
GROUPS_ALL = [[0, 1, 2, 3, 4, 5, 6, 7]]

_BUILT = {}


def _build(with_qk_bias):
    import concourse.bass as bass
    import concourse.tile as tile
    from concourse import bacc, mybir

    f32 = mybir.dt.float32
    bf16 = mybir.dt.bfloat16
    f16 = mybir.dt.float16
    AF = mybir.ActivationFunctionType
    OP = mybir.AluOpType

    nc = bacc.Bacc("TRN2", target_bir_lowering=False, debug=False,
                   num_devices=N_CORES)

    xT = nc.dram_tensor("xT", [2, 128, 8, S], bf16, kind="ExternalInput").ap()
    wq = nc.dram_tensor("wq", [128, 8, 128], bf16, kind="ExternalInput").ap()
    wk = nc.dram_tensor("wk", [128, 8, 128], bf16, kind="ExternalInput").ap()
    wv = nc.dram_tensor("wv", [128, 8, 128], bf16, kind="ExternalInput").ap()
    wo = nc.dram_tensor("wo", [128, 8, D], bf16, kind="ExternalInput").ap()
    cosd = nc.dram_tensor("cosT", [128, S], bf16, kind="ExternalInput").ap()
    sind = nc.dram_tensor("sinTm", [128, S], bf16, kind="ExternalInput").ap()
    ltd = nc.dram_tensor("ltm", [128, 2, 128], bf16, kind="ExternalInput").ap()
    if with_qk_bias:
        bqd = nc.dram_tensor("bq", [128, 1], f32, kind="ExternalInput").ap()
        bkd = nc.dram_tensor("bk", [128, 1], f32, kind="ExternalInput").ap()

    z_send = [nc.dram_tensor(f"z_send{j}", [8, 128, 128], bf16)
              for j in range(NCHUNK - 1)]
    z_recv = [nc.dram_tensor(f"z_recv{j}", [8, 128, 128], bf16)
              for j in range(NCHUNK - 1)]
    # final chunk exchanged as two column-half collectives so the second
    # mesh overlaps the first half's output projection
    z_sendh = [nc.dram_tensor(f"z_sendh{h}", [8, 128, 64], bf16)
               for h in range(2)]
    z_recvh = [nc.dram_tensor(f"z_recvh{h}", [8, 128, 64], bf16)
               for h in range(2)]
    out_ext = nc.dram_tensor("out_shard", [S // 4, D], f16,
                             kind="ExternalOutput").ap()

    with tile.TileContext(nc) as tc:
        with (
            tc.tile_pool(name="consts", bufs=1) as consts,
            tc.tile_pool(name="qk", bufs=1) as qkpool,
            tc.tile_pool(name="vsb", bufs=1) as vpool,
            tc.tile_pool(name="rope", bufs=2) as rope,
            tc.tile_pool(name="epool", bufs=2) as epool,
            tc.tile_pool(name="zpool", bufs=4) as zpool,
            tc.tile_pool(name="den", bufs=2) as den,
            tc.tile_pool(name="zail", bufs=2) as zallp,
            tc.tile_pool(name="osb", bufs=3) as osb,
            tc.tile_pool(name="ps_sc", bufs=2, space="PSUM") as ps_sc,
            tc.tile_pool(name="ps_av", bufs=2, space="PSUM") as ps_av,
            tc.tile_pool(name="ps_pj", bufs=2, space="PSUM") as ps_pj,
        ):
            # exp table load warm-up: very first instruction
            warm = consts.tile([128, 8], f32, tag="warm")
            nc.vector.memset(warm, 0.0)
            nc.scalar.activation(out=warm, in_=warm, func=AF.Exp, scale=1.0)

            # const DMAs on the gpsimd queue (issue only; ring runs async).
            # xT (0,0) kt-slices ride this ring too: its preamble ends ~3us
            # before the sync ring's, so the first proj matmul starts sooner.
            xT_sb = consts.tile([128, 2, 8, S], bf16, tag="xT")
            wq_sb = consts.tile([128, 8, 128], bf16, tag="wq")
            nc.gpsimd.dma_start(out=wq_sb, in_=wq)
            for kt in range(4):
                nc.gpsimd.dma_start(out=xT_sb[:, 0, kt, 0:QCHUNK],
                                    in_=xT[0][:, kt, 0:QCHUNK])
            wk_sb = consts.tile([128, 8, 128], bf16, tag="wk")
            nc.gpsimd.dma_start(out=wk_sb, in_=wk)
            for kt in range(4, 8):
                nc.gpsimd.dma_start(out=xT_sb[:, 0, kt, 0:QCHUNK],
                                    in_=xT[0][:, kt, 0:QCHUNK])
            cos_sb = consts.tile([128, S], bf16, tag="cos")
            nc.gpsimd.dma_start(out=cos_sb, in_=cosd)
            sin_sb = consts.tile([128, S], bf16, tag="sin")
            nc.gpsimd.dma_start(out=sin_sb, in_=sind)
            lt_sb = consts.tile([128, 2, 128], bf16, tag="ltm")
            nc.gpsimd.dma_start(out=lt_sb, in_=ltd)
            wv_sb = consts.tile([128, 8, 128], bf16, tag="wv")
            nc.gpsimd.dma_start(out=wv_sb, in_=wv)
            # wo (1 MB) is not needed until outproj(0) at stage 6: load it
            # on the sync ring after the startup-critical x chunks
            wo_sb = consts.tile([128, 8, D], bf16, tag="wo")
            if with_qk_bias:
                bq_sb = consts.tile([128, 1], f32, tag="bq")
                nc.gpsimd.dma_start(out=bq_sb, in_=bqd)
                bk_sb = consts.tile([128, 1], f32, tag="bk")
                nc.gpsimd.dma_start(out=bk_sb, in_=bkd)

            # x loads on the sync queue, in first-needed order, sliced so
            # proj kt-units start as soon as their k-slice lands (the 64MB
            # aggregate flood makes every transfer slow at startup).
            for (b, c) in [(1, 0), (0, 1)]:
                cs = slice(c * QCHUNK, (c + 1) * QCHUNK)
                for kt in range(8):
                    nc.sync.dma_start(out=xT_sb[:, b, kt, cs],
                                      in_=xT[b][:, kt, cs])
            cs = slice(QCHUNK, 2 * QCHUNK)
            for kh in range(2):
                nc.sync.dma_start(out=xT_sb[:, 1, 4 * kh:4 * kh + 4, cs],
                                  in_=xT[1][:, 4 * kh:4 * kh + 4, cs])
            nc.sync.dma_start(out=wo_sb, in_=wo)
            for (b, c) in [(0, 2), (1, 2), (0, 3), (1, 3)]:
                cs = slice(c * QCHUNK, (c + 1) * QCHUNK)
                for kh in range(2):
                    nc.sync.dma_start(
                        out=xT_sb[:, b, 4 * kh:4 * kh + 4, cs],
                        in_=xT[b][:, 4 * kh:4 * kh + 4, cs])

            # Persistent rotated Q/K: [128 (=2-head pack), batch, s]
            Qr = qkpool.tile([128, 2, S], bf16, tag="Qr")
            Kr = qkpool.tile([128, 2, S], bf16, tag="Kr")
            # V with ones column: [s-part, batch, s-tile, head, 65]
            Vs = vpool.tile([128, 2, 16, 2, 65], bf16, tag="Vs")
            nc.vector.memset(Vs[:, :, :, :, 64:65], 1.0)

            # ---------------- unit helpers ----------------
            qsb_state = {}    # (w, b, c) -> q_sb tile awaiting rope
            av_state = {}     # (b, j, hh) -> z psum tile
            zsb_state = {}    # (b, j) -> normalized z sbuf tile
            zall_state = {}   # j -> zall sbuf tile

            def projqk_half(w, b, c, half):
                wsb = wq_sb if w == "q" else wk_sb
                cs = slice(c * QCHUNK, (c + 1) * QCHUNK)

                def fn():
                    if half == 0:
                        pt = ps_pj.tile([128, QCHUNK], f32, tag="pjg",
                                        bufs=1)
                        qsb_state[("pt", w, b, c)] = pt
                    else:
                        pt = qsb_state.pop(("pt", w, b, c))
                    for kt in range(4 * half, 4 * half + 4):
                        nc.tensor.matmul(
                            out=pt, lhsT=wsb[:, kt, :],
                            rhs=xT_sb[:, b, kt, cs],
                            start=(kt == 0), stop=(kt == 7),
                            skip_group_check=True)
                    if half == 1:
                        if with_qk_bias:
                            bsb = bq_sb if w == "q" else bk_sb
                            nc.vector.tensor_scalar_add(
                                out=pt, in0=pt, scalar1=bsb[:, 0:1])
                        q_sb = rope.tile([128, QCHUNK], bf16, tag="ropeA")
                        nc.vector.tensor_copy(out=q_sb, in_=pt)
                        qsb_state[(w, b, c)] = q_sb
                return fn

            def rope_unit(w, b, c):
                dst = Qr if w == "q" else Kr
                cs = slice(c * QCHUNK, (c + 1) * QCHUNK)

                def fn():
                    q_sb = qsb_state.pop((w, b, c))
                    # q_rot = q*cos + flip(q)*sin' (sign folded into sin')
                    qf = rope.tile([128, QCHUNK], bf16, tag="ropeB")
                    for blk in range(4):
                        src = (blk ^ 1) * 32
                        nc.vector.tensor_copy(
                            out=qf[blk * 32:blk * 32 + 32, :],
                            in_=q_sb[src:src + 32, :])
                    qs = rope.tile([128, QCHUNK], bf16, tag="ropeC")
                    nc.vector.tensor_tensor(
                        out=qs, in0=qf, in1=sin_sb[:, cs], op=OP.mult)
                    qc = rope.tile([128, QCHUNK], bf16, tag="ropeB")
                    nc.vector.tensor_tensor(
                        out=qc, in0=q_sb, in1=cos_sb[:, cs], op=OP.mult)
                    nc.vector.tensor_tensor(
                        out=dst[:, b, cs], in0=qc, in1=qs, op=OP.add)
                return fn

            def projv_unit(b, c, st):
                def fn():
                    pt = ps_pj.tile([128, 2, 64], f32, tag="pjm", bufs=1)
                    for kt in range(8):
                        nc.tensor.matmul(
                            out=pt,
                            lhsT=xT_sb[:, b, kt, st * 128:(st + 1) * 128],
                            rhs=wv_sb[:, kt, :],
                            start=(kt == 0), stop=(kt == 7),
                            skip_group_check=True)
                    nc.vector.tensor_copy(out=Vs[:, b, st, :, 0:64], in_=pt)
                return fn

            def av_unit(b, j, hh, p, E):
                nkt = 4 * j + 4

                def fn():
                    if p == 0:
                        z = ps_av.tile([65, 4, 128], f32, tag="av")
                        av_state[(b, j, hh)] = z
                    else:
                        z = av_state[(b, j, hh)]
                    for t in (2 * p, 2 * p + 1):
                        q0 = max(0, 128 * (t - 4 * j))
                        nc.tensor.matmul(
                            out=z[:, q0 // 128:, :], lhsT=Vs[:, b, t, hh, :],
                            rhs=E[:, t, hh, q0:],
                            start=(t == 0), stop=(t == nkt - 1),
                            skip_group_check=True)
                return fn

            def norm_unit(b, j, hh):
                def fn():
                    z = av_state.pop((b, j, hh))
                    if hh == 0:
                        zsb = zpool.tile([128, 4, 128], bf16, tag="zsb")
                        zsb_state[(b, j)] = zsb
                    else:
                        zsb = zsb_state[(b, j)]
                    hs = slice(64 * hh, 64 * hh + 64)
                    d0 = den.tile([1, 4, 128], f32, tag="d0")
                    nc.vector.tensor_copy(out=d0, in_=z[64:65, :, :])
                    nc.vector.reciprocal_approx_fast(out=d0, in_=d0)
                    rb = den.tile([64, 4, 128], f32, tag="rb")
                    nc.gpsimd.partition_broadcast(out_ap=rb, in_ap=d0)
                    nc.vector.tensor_tensor(
                        out=zsb[hs, :, :], in0=z[0:64, :, :], in1=rb,
                        op=OP.mult)
                return fn

            def send_unit(b, j):
                def fn():
                    zsb = zsb_state.pop((b, j))
                    # DRAM-side dim permutation: out iterates (p, s4, c) to
                    # match the SBUF tile's natural partition-major order
                    if j < NCHUNK - 1:
                        nc.gpsimd.dma_start(
                            out=z_send[j].ap()[4 * b:4 * b + 4].transpose(
                                [1, 0, 2]),
                            in_=zsb)
                    else:
                        for h in range(2):
                            nc.gpsimd.dma_start(
                                out=z_sendh[h].ap()[4 * b:4 * b + 4]
                                .transpose([1, 0, 2]),
                                in_=zsb[:, :, 64 * h:64 * h + 64])
                return fn

            def trigger_unit(j, h=None):
                def fn():
                    if h is None:
                        nc.gpsimd.collective_compute(
                            "AllToAll", mybir.AluOpType.bypass,
                            replica_groups=GROUPS_ALL,
                            ins=[z_send[j].ap().opt()],
                            outs=[z_recv[j].ap().opt()])
                    else:
                        nc.gpsimd.collective_compute(
                            "AllToAll", mybir.AluOpType.bypass,
                            replica_groups=GROUPS_ALL,
                            ins=[z_sendh[h].ap().opt()],
                            outs=[z_recvh[h].ap().opt()])
                return fn

            def zall_unit(j, h=None):
                def fn():
                    if h is None or h == 0:
                        zall = zallp.tile([128, 8, 128], bf16, tag="zall")
                        zall_state[j] = zall
                    else:
                        zall = zall_state[j]
                    if h is None:
                        nc.sync.dma_start(
                            out=zall,
                            in_=z_recv[j].ap().transpose([1, 0, 2]))
                    else:
                        nc.sync.dma_start(
                            out=zall[:, :, 64 * h:64 * h + 64],
                            in_=z_recvh[h].ap().transpose([1, 0, 2]))
                return fn

            def outproj_half(j, mc, tag="pjm", qh=None):
                qs = slice(0, 128) if qh is None else \
                    slice(64 * qh, 64 * qh + 64)
                nq = 128 if qh is None else 64

                def fn():
                    zall = zall_state[j]
                    po = ps_pj.tile([nq, 512], f32, tag=tag, bufs=1)
                    for kt in range(8):
                        nc.tensor.matmul(
                            out=po, lhsT=zall[:, kt, qs],
                            rhs=wo_sb[:, kt, mc * 512:(mc + 1) * 512],
                            start=(kt == 0), stop=(kt == 7),
                            skip_group_check=True)
                    o_sb = osb.tile([nq, 512], f16, tag="osb")
                    nc.vector.tensor_copy(out=o_sb, in_=po)
                    nc.sync.dma_start(
                        out=out_ext[j * 128 + qs.start:
                                    j * 128 + qs.stop,
                                    mc * 512:(mc + 1) * 512],
                        in_=o_sb)
                return fn

            def proj_units(b, c):
                # interleave the pjg users (qk halves) with pjm users
                # (V s-tiles) so each pool's WAR-on-cast latency is hidden.
                # key=(b,c) marks units that stage (b,c)'s scores depend on.
                sts = list(range(4 * c, 4 * c + 4))
                us = [
                    (0, 1.05, projqk_half("q", b, c, 0), (b, c)),
                    (0, 1.05, projqk_half("q", b, c, 1), (b, c)),
                    (0, 0.05, rope_unit("q", b, c), (b, c)),
                    (0, 0.55, projv_unit(b, c, sts[0]), (b, c)),
                    (0, 1.05, projqk_half("k", b, c, 0), (b, c)),
                    (0, 0.55, projv_unit(b, c, sts[1]), (b, c)),
                    (0, 1.05, projqk_half("k", b, c, 1), (b, c)),
                    (0, 0.05, rope_unit("k", b, c), (b, c)),
                    (0, 0.55, projv_unit(b, c, sts[2]), (b, c)),
                    (0, 0.55, projv_unit(b, c, sts[3]), (b, c)),
                ]
                return us

            # ---------------- stage driver ----------------
            units = []  # (gate_tile, cost_us, fn, key)

            def drain(t, budget):
                while True:
                    idx = None
                    for i, (g, cst, fn, key) in enumerate(units):
                        if g > t:
                            continue  # gated: may be jumped (independent)
                        if cst <= budget + 1.2:
                            idx = i
                        break  # first READY unit pops or blocks the drain
                    if idx is None:
                        return budget
                    g, cst, fn, key = units.pop(idx)
                    fn()
                    budget -= cst

            def run_stage(b, j):
                nkt = 4 * j + 4
                # force-drain any projection units this stage's scores
                # depend on (Qr/Kr/Vs writers for (b, <=j)) — their writes
                # must be ISSUED before the first score matmul reads them
                last = max((i for i, u in enumerate(units)
                            if u[3] is not None and u[3][0] == b
                            and u[3][1] <= j), default=-1)
                for _ in range(last + 1):
                    g, cst, fn, key = units.pop(0)
                    fn()
                # units carried over from earlier stages have stale gates;
                # all their deps are already issued, so make them ready now
                units[:] = [(0, cst, fn, key)
                            for (_, cst, fn, key) in units]
                E = epool.tile([128, 16, 2, QCHUNK], bf16, tag="E")
                # this stage's AV + normalize + send, gated on exp progress
                for p in range(nkt // 2):
                    units.append((2 * p + 4, 0.55,
                                  av_unit(b, j, 0, p, E), None))
                    units.append((2 * p + 5, 0.55,
                                  av_unit(b, j, 1, p, E), None))
                units.append((nkt, 0.05, norm_unit(b, j, 0), None))
                units.append((nkt, 0.05, norm_unit(b, j, 1), None))
                units.append((nkt, 0.05, send_unit(b, j), None))
                # trigger right after the batch-1 send (chunk 0 deferred:
                # its sends crawl behind the startup HBM flood)
                if b == 1 and j >= 1:
                    if j < NCHUNK - 1:
                        units.append((nkt, 0.05, trigger_unit(j), None))
                    else:
                        units.append((nkt, 0.05, trigger_unit(j, 0), None))
                        units.append((nkt, 0.05, trigger_unit(j, 1), None))

                budget = 0.0
                for t in range(nkt):
                    q0 = max(0, 128 * (t - 4 * j))
                    qs2 = slice(j * QCHUNK + q0, (j + 1) * QCHUNK)
                    sc = ps_sc.tile([128, 2, QCHUNK], f32, tag="sc")
                    for hh in range(2):
                        hs = slice(64 * hh, 64 * hh + 64)
                        nc.tensor.matmul(
                            out=sc[:, hh, q0:],
                            lhsT=Kr[hs, b, t * 128:(t + 1) * 128],
                            rhs=Qr[hs, b, qs2], start=True, stop=True)
                    nc.scalar.activation(
                        out=E[:, t, :, q0:], in_=sc[:, :, q0:],
                        func=AF.Exp, scale=1.0 / SCALE)
                    if t >= 4 * j:  # diagonal tile: causal mask (q >= k)
                        qb = slice(q0, q0 + 128)
                        nc.vector.tensor_tensor(
                            out=E[:, t, :, qb], in0=E[:, t, :, qb],
                            in1=lt_sb, op=OP.mult)
                    budget += 0.55 * (QCHUNK - q0) / QCHUNK
                    budget = drain(t, budget)
                # leftovers carry into the next stage's burst, so the PE
                # never sits through a serial stage-end drain

            # ---------------- schedule ----------------
            # stage 0 (b=0, j=0): its own projections run inline first
            for g, cst, fn, key in proj_units(0, 0):
                fn()
            stages = [(b, j) for j in range(NCHUNK) for b in range(2)]
            for idx, (b, j) in enumerate(stages):
                # chunk 0's trigger one stage after its sends, so the
                # gpsimd queue never blocks on flood-era send DMAs
                if idx == 2:
                    units.append((4, 0.05, trigger_unit(0), None))
                # outproj(0)/(1) in stage 7 only: the early AllToAlls crawl
                # behind the 64MB x-load flood plus inter-core skew, and a
                # premature outproj matmul blocks the in-order PE queue
                if idx == 7:
                    units.append((1, 0.10, zall_unit(0), None))
                    units.append((2, 1.15, outproj_half(0, 0), None))
                    units.append((3, 1.15, outproj_half(0, 1), None))
                    units.append((4, 0.10, zall_unit(1), None))
                    units.append((5, 1.15, outproj_half(1, 0), None))
                    units.append((6, 1.15, outproj_half(1, 1), None))
                # projections for upcoming chunks
                if idx == 0:
                    units.extend(proj_units(1, 0))
                    units.extend(proj_units(0, 1))
                elif j < NCHUNK - 1:
                    units.extend(proj_units(b, j + 1))
                run_stage(b, j)
            while units:  # final stage's AV tail, norms, sends
                g, cst, fn, key = units.pop(0)
                fn()

            # epilogue: cover the final AllToAll (triggered in the last
            # stage's drain) with outproj(NCHUNK-2); then the last chunk's
            # first q-half projects while the second half's mesh finishes.
            zall_unit(NCHUNK - 2)()
            outproj_half(NCHUNK - 2, 0, tag="pjg")()
            outproj_half(NCHUNK - 2, 1)()
            jl = NCHUNK - 1
            zall_unit(jl, 0)()
            outproj_half(jl, 0, tag="pjg", qh=0)()
            outproj_half(jl, 1, tag="pjm", qh=0)()
            zall_unit(jl, 1)()
            outproj_half(jl, 0, tag="pjg", qh=1)()
            outproj_half(jl, 1, tag="pjm", qh=1)()

    nc.compile()
    return nc


def _get_built(with_qk_bias):
    key = bool(with_qk_bias)
    if key not in _BUILT:
        _BUILT[key] = _build(key)
    return _BUILT[key]


def _rope_tables():
    pos = np.arange(S, dtype=np.float64)
    dim = np.arange(DH // 2, dtype=np.float64)
    freq = ROT_BASE ** (dim / (DH / 2))
    freq = np.concatenate([freq, freq])                # [64]
    ang = pos[None, :] / freq[:, None]                 # [64, S]
    cos = np.cos(ang)
    sin = np.sin(ang)
    # sign of the rotate-half term folded into sin': rows 0..31 get -sin
    sinm = sin.copy()
    sinm[:DH // 2] *= -1.0
    cosT = np.tile(cos, (2, 1)).astype(BF)             # [128, S]
    sinT = np.tile(sinm, (2, 1)).astype(BF)
    return cosT, sinT


def kernel(x, W_Q, b_Q, W_K, b_K, W_V, b_V, W_O, b_O):
    from concourse.bass_utils import run_bass_kernel_spmd

    x = np.asarray(x)
    W_Q, W_K, W_V, W_O = (np.asarray(a) for a in (W_Q, W_K, W_V, W_O))
    b_Q, b_K, b_V, b_O = (np.asarray(a) for a in (b_Q, b_K, b_V, b_O))
    with_qk_bias = bool(np.any(b_Q) or np.any(b_K))
    nc = _get_built(with_qk_bias)

    cosT, sinT = _rope_tables()
    lt = np.tril(np.ones((128, 128), dtype=np.float32)).T  # [k, q]: q >= k
    ltm = np.ascontiguousarray(
        np.broadcast_to(lt[:, None, :], (128, 2, 128))).astype(BF)

    def wtile(w):            # [1024, C] -> [128, 8, C]
        c = w.shape[1]
        return np.ascontiguousarray(
            w.reshape(8, 128, c).transpose(1, 0, 2)).astype(BF)

    # x transposed per batch: [d, s]: d = kt*128 + p -> [p, kt, s]
    xT_host = np.stack([
        np.ascontiguousarray(
            x[b].T.reshape(8, 128, S).transpose(1, 0, 2)).astype(BF)
        for b in range(2)], axis=0)
    # W_O for ALL heads: slot s = heads (2s, 2s+1); identical on all cores
    wo_h = np.ascontiguousarray(
        np.concatenate([W_O[h] for h in range(NH)], axis=0)  # [1024, 1024]
        .reshape(8, 128, D).transpose(1, 0, 2)).astype(BF)

    in_maps = []
    for core in range(N_CORES):
        h0 = 2 * core
        wq_h = wtile(np.concatenate([W_Q[h0], W_Q[h0 + 1]], axis=1))
        wk_h = wtile(np.concatenate([W_K[h0], W_K[h0 + 1]], axis=1))
        wv_h = wtile(np.concatenate([W_V[h0], W_V[h0 + 1]], axis=1))
        m = {
            "xT": xT_host, "wq": wq_h, "wk": wk_h, "wv": wv_h, "wo": wo_h,
            "cosT": cosT, "sinTm": sinT, "ltm": ltm,
        }
        if with_qk_bias:
            m["bq"] = np.concatenate(
                [b_Q[h0], b_Q[h0 + 1]]).astype(np.float32)[:, None]
            m["bk"] = np.concatenate(
                [b_K[h0], b_K[h0 + 1]]).astype(np.float32)[:, None]
        in_maps.append(m)

    global _last_in_maps
    _last_in_maps = in_maps
    res = run_bass_kernel_spmd(nc, in_maps, list(range(N_CORES)))

    out = np.empty((2, S, D), dtype=np.float32)
    for core in range(N_CORES):
        b, r = divmod(core, 4)
        shard = res.results[core]["out_shard"].astype(np.float32)
        for j in range(NCHUNK):
            out[b, QCHUNK * j + 128 * r: QCHUNK * j + 128 * (r + 1), :] = \
                shard[128 * j:128 * (j + 1)]

    # b_V shifts z by exactly b_V (softmax rows sum to 1); fold with b_O.
    corr = b_O.astype(np.float64).copy()
    if np.any(b_V):
        corr = corr + np.einsum("hd,hdm->m", b_V.astype(np.float64),
                                W_O.astype(np.float64))
    if np.any(corr):
        out = out + corr.astype(np.float32)
    return out


# revision 66
# speedup vs baseline: 1.4426x; 1.4426x over previous
"""Trainium2 Bass kernel for causal multi-head attention with NeoX RoPE.

Problem: x[2, 2048, 1024], 16 heads x d_head 64, rotary over all 64 dims,
causal softmax, output projection.

Sharding: every core holds 2 heads ({2c, 2c+1}) and processes BOTH
batches.  After a per-q-chunk 8-core AllToAll of the normalized z shards,
each core contracts all 16 heads locally and writes its own [128 x 1024]
output rows per chunk.

v3 scheduling: all work is decomposed into ~0.5-1.1us units (proj kt
halves, V-proj per s-tile, AV 2-ktile groups, outproj halves, rope,
normalize, DMA) drained between score tiles with a cost budget, so the
PE never idles while exp paces the score psum recycle.  AV of a stage
runs inside the SAME stage's burst (gated on exp progress), which keeps
the tail short.  Collective-chain DMAs (z_send/zall/out) live on the
Sync queue; GpSimd only does affine_select + partition_broadcast, so the
collective wait never head-of-line-blocks the causal mask.  The final
chunk's outproj(2) is reserved to overlap the last AllToAll.
"""

import numpy as np
import ml_dtypes

S = 2048
D = 1024
NH = 16
DH = 64
SCALE = 8.0
ROT_BASE = 10000.0
N_CORES = 8
QCHUNK = 512     # q chunk (free dim of score matmuls)
NCHUNK = S // QCHUNK
KTILE = 128
BF = ml_dtypes.bfloat16
GROUPS_ALL = [[0, 1, 2, 3, 4, 5, 6, 7]]

_BUILT = {}


def _build(with_qk_bias):
    import concourse.bass as bass
    import concourse.tile as tile
    from concourse import bacc, mybir

    f32 = mybir.dt.float32
    bf16 = mybir.dt.bfloat16
    f16 = mybir.dt.float16
    AF = mybir.ActivationFunctionType
    OP = mybir.AluOpType

    nc = bacc.Bacc("TRN2", target_bir_lowering=False, debug=False,
                   num_devices=N_CORES)

    xT = nc.dram_tensor("xT", [2, 128, 8, S], bf16, kind="ExternalInput").ap()
    wq = nc.dram_tensor("wq", [128, 8, 128], bf16, kind="ExternalInput").ap()
    wk = nc.dram_tensor("wk", [128, 8, 128], bf16, kind="ExternalInput").ap()
    wv = nc.dram_tensor("wv", [128, 8, 128], bf16, kind="ExternalInput").ap()
    wo = nc.dram_tensor("wo", [128, 8, D], bf16, kind="ExternalInput").ap()
    cosd = nc.dram_tensor("cosT", [128, S], bf16, kind="ExternalInput").ap()
    sind = nc.dram_tensor("sinTm", [128, S], bf16, kind="ExternalInput").ap()
    ltd = nc.dram_tensor("ltm", [128, 2, 128], bf16, kind="ExternalInput").ap()
    if with_qk_bias:
        bqd = nc.dram_tensor("bq", [128, 1], f32, kind="ExternalInput").ap()
        bkd = nc.dram_tensor("bk", [128, 1], f32, kind="ExternalInput").ap()

    z_send = [nc.dram_tensor(f"z_send{j}", [8, 128, 128], bf16)
              for j in range(NCHUNK)]
    z_recv = [nc.dram_tensor(f"z_recv{j}", [8, 128, 128], bf16)
              for j in range(NCHUNK)]
    out_ext = nc.dram_tensor("out_shard", [S // 4, D], f16,
                             kind="ExternalOutput").ap()

    with tile.TileContext(nc) as tc:
        with (
            tc.tile_pool(name="consts", bufs=1) as consts,
            tc.tile_pool(name="qk", bufs=1) as qkpool,
            tc.tile_pool(name="vsb", bufs=1) as vpool,
            tc.tile_pool(name="rope", bufs=2) as rope,
            tc.tile_pool(name="epool", bufs=2) as epool,
            tc.tile_pool(name="zpool", bufs=4) as zpool,
            tc.tile_pool(name="den", bufs=2) as den,
            tc.tile_pool(name="zail", bufs=2) as zallp,
            tc.tile_pool(name="osb", bufs=3) as osb,
            tc.tile_pool(name="ps_sc", bufs=2, space="PSUM") as ps_sc,
            tc.tile_pool(name="ps_av", bufs=2, space="PSUM") as ps_av,
            tc.tile_pool(name="ps_pj", bufs=2, space="PSUM") as ps_pj,
        ):
            # exp table load warm-up: very first instruction
            warm = consts.tile([128, 8], f32, tag="warm")
            nc.vector.memset(warm, 0.0)
            nc.scalar.activation(out=warm, in_=warm, func=AF.Exp, scale=1.0)

            # const DMAs on the gpsimd queue (issue only; ring runs async).
            # xT (0,0) kt-slices ride this ring too: its preamble ends ~3us
            # before the sync ring's, so the first proj matmul starts sooner.
            xT_sb = consts.tile([128, 2, 8, S], bf16, tag="xT")
            wq_sb = consts.tile([128, 8, 128], bf16, tag="wq")
            nc.gpsimd.dma_start(out=wq_sb, in_=wq)
            for kt in range(4):
                nc.gpsimd.dma_start(out=xT_sb[:, 0, kt, 0:QCHUNK],
                                    in_=xT[0][:, kt, 0:QCHUNK])
            wk_sb = consts.tile([128, 8, 128], bf16, tag="wk")
            nc.gpsimd.dma_start(out=wk_sb, in_=wk)
            for kt in range(4, 8):
                nc.gpsimd.dma_start(out=xT_sb[:, 0, kt, 0:QCHUNK],
                                    in_=xT[0][:, kt, 0:QCHUNK])
            cos_sb = consts.tile([128, S], bf16, tag="cos")
            nc.gpsimd.dma_start(out=cos_sb, in_=cosd)
            sin_sb = consts.tile([128, S], bf16, tag="sin")
            nc.gpsimd.dma_start(out=sin_sb, in_=sind)
            lt_sb = consts.tile([128, 2, 128], bf16, tag="ltm")
            nc.gpsimd.dma_start(out=lt_sb, in_=ltd)
            wv_sb = consts.tile([128, 8, 128], bf16, tag="wv")
            nc.gpsimd.dma_start(out=wv_sb, in_=wv)
            # wo (1 MB) is not needed until outproj(0) at stage 6: load it
            # on the sync ring after the startup-critical x chunks
            wo_sb = consts.tile([128, 8, D], bf16, tag="wo")
            if with_qk_bias:
                bq_sb = consts.tile([128, 1], f32, tag="bq")
                nc.gpsimd.dma_start(out=bq_sb, in_=bqd)
                bk_sb = consts.tile([128, 1], f32, tag="bk")
                nc.gpsimd.dma_start(out=bk_sb, in_=bkd)

            # x loads on the sync queue, in first-needed order, sliced so
            # proj kt-units start as soon as their k-slice lands (the 64MB
            # aggregate flood makes every transfer slow at startup).
            for (b, c) in [(1, 0), (0, 1)]:
                cs = slice(c * QCHUNK, (c + 1) * QCHUNK)
                for kt in range(8):
                    nc.sync.dma_start(out=xT_sb[:, b, kt, cs],
                                      in_=xT[b][:, kt, cs])
            cs = slice(QCHUNK, 2 * QCHUNK)
            for kh in range(2):
                nc.sync.dma_start(out=xT_sb[:, 1, 4 * kh:4 * kh + 4, cs],
                                  in_=xT[1][:, 4 * kh:4 * kh + 4, cs])
            nc.sync.dma_start(out=wo_sb, in_=wo)
            for (b, c) in [(0, 2), (1, 2), (0, 3), (1, 3)]:
                cs = slice(c * QCHUNK, (c + 1) * QCHUNK)
                for kh in range(2):
                    nc.sync.dma_start(
                        out=xT_sb[:, b, 4 * kh:4 * kh + 4, cs],
                        in_=xT[b][:, 4 * kh:4 * kh + 4, cs])

            # Persistent rotated Q/K: [128 (=2-head pack), batch, s]
            Qr = qkpool.tile([128, 2, S], bf16, tag="Qr")
            Kr = qkpool.tile([128, 2, S], bf16, tag="Kr")
            # V with ones column: [s-part, batch, s-tile, head, 65]
            Vs = vpool.tile([128, 2, 16, 2, 65], bf16, tag="Vs")
            nc.vector.memset(Vs[:, :, :, :, 64:65], 1.0)

            # ---------------- unit helpers ----------------
            qsb_state = {}    # (w, b, c) -> q_sb tile awaiting rope
            av_state = {}     # (b, j, hh) -> z psum tile
            zsb_state = {}    # (b, j) -> normalized z sbuf tile
            zall_state = {}   # j -> zall sbuf tile

            def projqk_half(w, b, c, half):
                wsb = wq_sb if w == "q" else wk_sb
                cs = slice(c * QCHUNK, (c + 1) * QCHUNK)

                def fn():
                    if half == 0:
                        pt = ps_pj.tile([128, QCHUNK], f32, tag="pjg",
                                        bufs=1)
                        qsb_state[("pt", w, b, c)] = pt
                    else:
                        pt = qsb_state.pop(("pt", w, b, c))
                    for kt in range(4 * half, 4 * half + 4):
                        nc.tensor.matmul(
                            out=pt, lhsT=wsb[:, kt, :],
                            rhs=xT_sb[:, b, kt, cs],
                            start=(kt == 0), stop=(kt == 7),
                            skip_group_check=True)
                    if half == 1:
                        if with_qk_bias:
                            bsb = bq_sb if w == "q" else bk_sb
                            nc.vector.tensor_scalar_add(
                                out=pt, in0=pt, scalar1=bsb[:, 0:1])
                        q_sb = rope.tile([128, QCHUNK], bf16, tag="ropeA")
                        nc.vector.tensor_copy(out=q_sb, in_=pt)
                        qsb_state[(w, b, c)] = q_sb
                return fn

            def rope_unit(w, b, c):
                dst = Qr if w == "q" else Kr
                cs = slice(c * QCHUNK, (c + 1) * QCHUNK)

                def fn():
                    q_sb = qsb_state.pop((w, b, c))
                    # q_rot = q*cos + flip(q)*sin' (sign folded into sin')
                    qf = rope.tile([128, QCHUNK], bf16, tag="ropeB")
                    for blk in range(4):
                        src = (blk ^ 1) * 32
                        nc.vector.tensor_copy(
                            out=qf[blk * 32:blk * 32 + 32, :],
                            in_=q_sb[src:src + 32, :])
                    qs = rope.tile([128, QCHUNK], bf16, tag="ropeC")
                    nc.vector.tensor_tensor(
                        out=qs, in0=qf, in1=sin_sb[:, cs], op=OP.mult)
                    qc = rope.tile([128, QCHUNK], bf16, tag="ropeB")
                    nc.vector.tensor_tensor(
                        out=qc, in0=q_sb, in1=cos_sb[:, cs], op=OP.mult)
                    nc.vector.tensor_tensor(
                        out=dst[:, b, cs], in0=qc, in1=qs, op=OP.add)
                return fn

            def projv_unit(b, c, st):
                def fn():
                    pt = ps_pj.tile([128, 2, 64], f32, tag="pjm", bufs=1)
                    for kt in range(8):
                        nc.tensor.matmul(
                            out=pt,
                            lhsT=xT_sb[:, b, kt, st * 128:(st + 1) * 128],
                            rhs=wv_sb[:, kt, :],
                            start=(kt == 0), stop=(kt == 7),
                            skip_group_check=True)
                    nc.vector.tensor_copy(out=Vs[:, b, st, :, 0:64], in_=pt)
                return fn

            def av_unit(b, j, hh, p, E):
                nkt = 4 * j + 4

                def fn():
                    if p == 0:
                        z = ps_av.tile([65, 4, 128], f32, tag="av")
                        av_state[(b, j, hh)] = z
                    else:
                        z = av_state[(b, j, hh)]
                    for t in (2 * p, 2 * p + 1):
                        q0 = max(0, 128 * (t - 4 * j))
                        nc.tensor.matmul(
                            out=z[:, q0 // 128:, :], lhsT=Vs[:, b, t, hh, :],
                            rhs=E[:, t, hh, q0:],
                            start=(t == 0), stop=(t == nkt - 1),
                            skip_group_check=True)
                return fn

            def norm_unit(b, j, hh):
                def fn():
                    z = av_state.pop((b, j, hh))
                    if hh == 0:
                        zsb = zpool.tile([128, 4, 128], bf16, tag="zsb")
                        zsb_state[(b, j)] = zsb
                    else:
                        zsb = zsb_state[(b, j)]
                    hs = slice(64 * hh, 64 * hh + 64)
                    d0 = den.tile([1, 4, 128], f32, tag="d0")
                    nc.vector.tensor_copy(out=d0, in_=z[64:65, :, :])
                    nc.vector.reciprocal_approx_fast(out=d0, in_=d0)
                    rb = den.tile([64, 4, 128], f32, tag="rb")
                    nc.gpsimd.partition_broadcast(out_ap=rb, in_ap=d0)
                    nc.vector.tensor_tensor(
                        out=zsb[hs, :, :], in0=z[0:64, :, :], in1=rb,
                        op=OP.mult)
                return fn

            def send_unit(b, j):
                def fn():
                    zsb = zsb_state.pop((b, j))
                    # DRAM-side dim permutation: out iterates (p, s4, c) to
                    # match the SBUF tile's natural partition-major order
                    nc.gpsimd.dma_start(
                        out=z_send[j].ap()[4 * b:4 * b + 4].transpose(
                            [1, 0, 2]),
                        in_=zsb)
                return fn

            def trigger_unit(j):
                def fn():
                    nc.gpsimd.collective_compute(
                        "AllToAll", mybir.AluOpType.bypass,
                        replica_groups=GROUPS_ALL,
                        ins=[z_send[j].ap().opt()],
                        outs=[z_recv[j].ap().opt()])
                return fn

            def zall_unit(j):
                def fn():
                    zall = zallp.tile([128, 8, 128], bf16, tag="zall")
                    zall_state[j] = zall
                    nc.sync.dma_start(
                        out=zall,
                        in_=z_recv[j].ap().transpose([1, 0, 2]))
                return fn

            def outproj_half(j, mc, tag="pjm"):
                def fn():
                    zall = zall_state[j]
                    po = ps_pj.tile([128, 512], f32, tag=tag, bufs=1)
                    for kt in range(8):
                        nc.tensor.matmul(
                            out=po, lhsT=zall[:, kt, :],
                            rhs=wo_sb[:, kt, mc * 512:(mc + 1) * 512],
                            start=(kt == 0), stop=(kt == 7),
                            skip_group_check=True)
                    o_sb = osb.tile([128, 512], f16, tag="osb")
                    nc.vector.tensor_copy(out=o_sb, in_=po)
                    nc.sync.dma_start(
                        out=out_ext[j * 128:(j + 1) * 128,
                                    mc * 512:(mc + 1) * 512],
                        in_=o_sb)
                return fn

            def proj_units(b, c):
                # interleave the pjg users (qk halves) with pjm users
                # (V s-tiles) so each pool's WAR-on-cast latency is hidden.
                # key=(b,c) marks units that stage (b,c)'s scores depend on.
                sts = list(range(4 * c, 4 * c + 4))
                us = [
                    (0, 1.05, projqk_half("q", b, c, 0), (b, c)),
                    (0, 1.05, projqk_half("q", b, c, 1), (b, c)),
                    (0, 0.05, rope_unit("q", b, c), (b, c)),
                    (0, 0.55, projv_unit(b, c, sts[0]), (b, c)),
                    (0, 1.05, projqk_half("k", b, c, 0), (b, c)),
                    (0, 0.55, projv_unit(b, c, sts[1]), (b, c)),
                    (0, 1.05, projqk_half("k", b, c, 1), (b, c)),
                    (0, 0.05, rope_unit("k", b, c), (b, c)),
                    (0, 0.55, projv_unit(b, c, sts[2]), (b, c)),
                    (0, 0.55, projv_unit(b, c, sts[3]), (b, c)),
                ]
                return us

            # ---------------- stage driver ----------------
            units = []  # (gate_tile, cost_us, fn, key)

            def drain(t, budget):
                while True:
                    idx = None
                    for i, (g, cst, fn, key) in enumerate(units):
                        if g > t:
                            continue  # gated: may be jumped (independent)
                        if cst <= budget + 1.2:
                            idx = i
                        break  # first READY unit pops or blocks the drain
                    if idx is None:
                        return budget
                    g, cst, fn, key = units.pop(idx)
                    fn()
                    budget -= cst

            def run_stage(b, j):
                nkt = 4 * j + 4
                # force-drain any projection units this stage's scores
                # depend on (Qr/Kr/Vs writers for (b, <=j)) — their writes
                # must be ISSUED before the first score matmul reads them
                last = max((i for i, u in enumerate(units)
                            if u[3] is not None and u[3][0] == b
                            and u[3][1] <= j), default=-1)
                for _ in range(last + 1):
                    g, cst, fn, key = units.pop(0)
                    fn()
                # units carried over from earlier stages have stale gates;
                # all their deps are already issued, so make them ready now
                units[:] = [(0, cst, fn, key)
                            for (_, cst, fn, key) in units]
                E = epool.tile([128, 16, 2, QCHUNK], bf16, tag="E")
                # this stage's AV + normalize + send, gated on exp progress
                for p in range(nkt // 2):
                    units.append((2 * p + 4, 0.55,
                                  av_unit(b, j, 0, p, E), None))
                    units.append((2 * p + 5, 0.55,
                                  av_unit(b, j, 1, p, E), None))
                units.append((nkt, 0.05, norm_unit(b, j, 0), None))
                units.append((nkt, 0.05, norm_unit(b, j, 1), None))
                units.append((nkt, 0.05, send_unit(b, j), None))
                # trigger right after the batch-1 send (chunk 0 deferred:
                # its sends crawl behind the startup HBM flood)
                if b == 1 and j >= 1:
                    units.append((nkt, 0.05, trigger_unit(j), None))

                budget = 0.0
                for t in range(nkt):
                    q0 = max(0, 128 * (t - 4 * j))
                    qs2 = slice(j * QCHUNK + q0, (j + 1) * QCHUNK)
                    sc = ps_sc.tile([128, 2, QCHUNK], f32, tag="sc")
                    for hh in range(2):
                        hs = slice(64 * hh, 64 * hh + 64)
                        nc.tensor.matmul(
                            out=sc[:, hh, q0:],
                            lhsT=Kr[hs, b, t * 128:(t + 1) * 128],
                            rhs=Qr[hs, b, qs2], start=True, stop=True)
                    nc.scalar.activation(
                        out=E[:, t, :, q0:], in_=sc[:, :, q0:],
                        func=AF.Exp, scale=1.0 / SCALE)
                    if t >= 4 * j:  # diagonal tile: causal mask (q >= k)
                        qb = slice(q0, q0 + 128)
                        nc.vector.tensor_tensor(
                            out=E[:, t, :, qb], in0=E[:, t, :, qb],
                            in1=lt_sb, op=OP.mult)
                    budget += 0.55 * (QCHUNK - q0) / QCHUNK
                    budget = drain(t, budget)
                # leftovers carry into the next stage's burst, so the PE
                # never sits through a serial stage-end drain

            # ---------------- schedule ----------------
            # stage 0 (b=0, j=0): its own projections run inline first
            for g, cst, fn, key in proj_units(0, 0):
                fn()
            stages = [(b, j) for j in range(NCHUNK) for b in range(2)]
            for idx, (b, j) in enumerate(stages):
                # chunk 0's trigger one stage after its sends, so the
                # gpsimd queue never blocks on flood-era send DMAs
                if idx == 2:
                    units.append((4, 0.05, trigger_unit(0), None))
                # outproj(0)/(1) in stage 7 only: the early AllToAlls crawl
                # behind the 64MB x-load flood plus inter-core skew, and a
                # premature outproj matmul blocks the in-order PE queue
                if idx == 7:
                    units.append((1, 0.10, zall_unit(0), None))
                    units.append((2, 1.15, outproj_half(0, 0), None))
                    units.append((3, 1.15, outproj_half(0, 1), None))
                    units.append((4, 0.10, zall_unit(1), None))
                    units.append((5, 1.15, outproj_half(1, 0), None))
                    units.append((6, 1.15, outproj_half(1, 1), None))
                # projections for upcoming chunks
                if idx == 0:
                    units.extend(proj_units(1, 0))
                    units.extend(proj_units(0, 1))
                elif j < NCHUNK - 1:
                    units.extend(proj_units(b, j + 1))
                run_stage(b, j)
            while units:  # final stage's AV tail, norms, sends
                g, cst, fn, key = units.pop(0)
                fn()

            # epilogue: cover the final AllToAll (triggered in the last
            # stage's drain) with outproj(NCHUNK-2), then the last chunk.
            for jp in range(NCHUNK - 2, NCHUNK):
                zall_unit(jp)()
                outproj_half(jp, 0, tag="pjg")()
                outproj_half(jp, 1)()

    nc.compile()
    return nc


def _get_built(with_qk_bias):
    key = bool(with_qk_bias)
    if key not in _BUILT:
        _BUILT[key] = _build(key)
    return _BUILT[key]


def _rope_tables():
    pos = np.arange(S, dtype=np.float64)
    dim = np.arange(DH // 2, dtype=np.float64)
    freq = ROT_BASE ** (dim / (DH / 2))
    freq = np.concatenate([freq, freq])                # [64]
    ang = pos[None, :] / freq[:, None]                 # [64, S]
    cos = np.cos(ang)
    sin = np.sin(ang)
    # sign of the rotate-half term folded into sin': rows 0..31 get -sin
    sinm = sin.copy()
    sinm[:DH // 2] *= -1.0
    cosT = np.tile(cos, (2, 1)).astype(BF)             # [128, S]
    sinT = np.tile(sinm, (2, 1)).astype(BF)
    return cosT, sinT


def kernel(x, W_Q, b_Q, W_K, b_K, W_V, b_V, W_O, b_O):
    from concourse.bass_utils import run_bass_kernel_spmd

    x = np.asarray(x)
    W_Q, W_K, W_V, W_O = (np.asarray(a) for a in (W_Q, W_K, W_V, W_O))
    b_Q, b_K, b_V, b_O = (np.asarray(a) for a in (b_Q, b_K, b_V, b_O))
    with_qk_bias = bool(np.any(b_Q) or np.any(b_K))
    nc = _get_built(with_qk_bias)

    cosT, sinT = _rope_tables()
    lt = np.tril(np.ones((128, 128), dtype=np.float32)).T  # [k, q]: q >= k
    ltm = np.ascontiguousarray(
        np.broadcast_to(lt[:, None, :], (128, 2, 128))).astype(BF)

    def wtile(w):            # [1024, C] -> [128, 8, C]
        c = w.shape[1]
        return np.ascontiguousarray(
            w.reshape(8, 128, c).transpose(1, 0, 2)).astype(BF)

    # x transposed per batch: [d, s]: d = kt*128 + p -> [p, kt, s]
    xT_host = np.stack([
        np.ascontiguousarray(
            x[b].T.reshape(8, 128, S).transpose(1, 0, 2)).astype(BF)
        for b in range(2)], axis=0)
    # W_O for ALL heads: slot s = heads (2s, 2s+1); identical on all cores
    wo_h = np.ascontiguousarray(
        np.concatenate([W_O[h] for h in range(NH)], axis=0)  # [1024, 1024]
        .reshape(8, 128, D).transpose(1, 0, 2)).astype(BF)

    in_maps = []
    for core in range(N_CORES):
        h0 = 2 * core
        wq_h = wtile(np.concatenate([W_Q[h0], W_Q[h0 + 1]], axis=1))
        wk_h = wtile(np.concatenate([W_K[h0], W_K[h0 + 1]], axis=1))
        wv_h = wtile(np.concatenate([W_V[h0], W_V[h0 + 1]], axis=1))
        m = {
            "xT": xT_host, "wq": wq_h, "wk": wk_h, "wv": wv_h, "wo": wo_h,
            "cosT": cosT, "sinTm": sinT, "ltm": ltm,
        }
        if with_qk_bias:
            m["bq"] = np.concatenate(
                [b_Q[h0], b_Q[h0 + 1]]).astype(np.float32)[:, None]
            m["bk"] = np.concatenate(
                [b_K[h0], b_K[h0 + 1]]).astype(np.float32)[:, None]
        in_maps.append(m)

    global _last_in_maps
    _last_in_maps = in_maps
    res = run_bass_kernel_spmd(nc, in_maps, list(range(N_CORES)))

    out = np.empty((2, S, D), dtype=np.float32)
    for core in range(N_CORES):
        b, r = divmod(core, 4)
        shard = res.results[core]["out_shard"].astype(np.float32)
        for j in range(NCHUNK):
            out[b, QCHUNK * j + 128 * r: QCHUNK * j + 128 * (r + 1), :] = \
                shard[128 * j:128 * (j + 1)]

    # b_V shifts z by exactly b_V (softmax rows sum to 1); fold with b_O.
    corr = b_O.astype(np.float64).copy()
    if np.any(b_V):
        corr = corr + np.einsum("hd,hdm->m", b_V.astype(np.float64),
                                W_O.astype(np.float64))
    if np.any(corr):
        out = out + corr.astype(np.float32)
    return out


# revision 69
# speedup vs baseline: 1.4746x; 1.0221x over previous
"""Trainium2 Bass kernel for causal multi-head attention with NeoX RoPE.

Problem: x[2, 2048, 1024], 16 heads x d_head 64, rotary over all 64 dims,
causal softmax, output projection.

Sharding: every core holds 2 heads ({2c, 2c+1}) and processes BOTH
batches.  After a per-q-chunk 8-core AllToAll of the normalized z shards,
each core contracts all 16 heads locally and writes its own [128 x 1024]
output rows per chunk.

v3 scheduling: all work is decomposed into ~0.5-1.1us units (proj kt
halves, V-proj per s-tile, AV 2-ktile groups, outproj halves, rope,
normalize, DMA) drained between score tiles with a cost budget, so the
PE never idles while exp paces the score psum recycle.  AV of a stage
runs inside the SAME stage's burst (gated on exp progress), which keeps
the tail short.  Collective-chain DMAs (z_send/zall/out) live on the
Sync queue; GpSimd only does affine_select + partition_broadcast, so the
collective wait never head-of-line-blocks the causal mask.  The final
chunk's outproj(2) is reserved to overlap the last AllToAll.
"""

import numpy as np
import ml_dtypes

S = 2048
D = 1024
NH = 16
DH = 64
SCALE = 8.0
ROT_BASE = 10000.0
N_CORES = 8
QCHUNK = 512     # q chunk (free dim of score matmuls)
NCHUNK = S // QCHUNK
KTILE = 128
BF = ml_dtypes.bfloat16
GROUPS_ALL = [[0, 1, 2, 3, 4, 5, 6, 7]]

_BUILT = {}


def _build(with_qk_bias):
    import concourse.bass as bass
    import concourse.tile as tile
    from concourse import bacc, mybir

    f32 = mybir.dt.float32
    bf16 = mybir.dt.bfloat16
    f16 = mybir.dt.float16
    AF = mybir.ActivationFunctionType
    OP = mybir.AluOpType

    nc = bacc.Bacc("TRN2", target_bir_lowering=False, debug=False,
                   num_devices=N_CORES)

    xT = nc.dram_tensor("xT", [2, 128, 8, S], bf16, kind="ExternalInput").ap()
    wq = nc.dram_tensor("wq", [128, 8, 128], bf16, kind="ExternalInput").ap()
    wk = nc.dram_tensor("wk", [128, 8, 128], bf16, kind="ExternalInput").ap()
    wv = nc.dram_tensor("wv", [128, 8, 128], bf16, kind="ExternalInput").ap()
    wo = nc.dram_tensor("wo", [128, 8, D], bf16, kind="ExternalInput").ap()
    cosd = nc.dram_tensor("cosT", [128, S], bf16, kind="ExternalInput").ap()
    sind = nc.dram_tensor("sinTm", [128, S], bf16, kind="ExternalInput").ap()
    ltd = nc.dram_tensor("ltm", [128, 2, 128], bf16, kind="ExternalInput").ap()
    if with_qk_bias:
        bqd = nc.dram_tensor("bq", [128, 1], f32, kind="ExternalInput").ap()
        bkd = nc.dram_tensor("bk", [128, 1], f32, kind="ExternalInput").ap()

    z_send = [nc.dram_tensor(f"z_send{j}", [8, 128, 128], bf16)
              for j in range(NCHUNK)]
    z_recv = [nc.dram_tensor(f"z_recv{j}", [8, 128, 128], bf16)
              for j in range(NCHUNK)]
    out_ext = nc.dram_tensor("out_shard", [S // 4, D], f16,
                             kind="ExternalOutput").ap()

    with tile.TileContext(nc) as tc:
        with (
            tc.tile_pool(name="consts", bufs=1) as consts,
            tc.tile_pool(name="qk", bufs=1) as qkpool,
            tc.tile_pool(name="vsb", bufs=1) as vpool,
            tc.tile_pool(name="rope", bufs=2) as rope,
            tc.tile_pool(name="epool", bufs=2) as epool,
            tc.tile_pool(name="zpool", bufs=4) as zpool,
            tc.tile_pool(name="den", bufs=2) as den,
            tc.tile_pool(name="zail", bufs=2) as zallp,
            tc.tile_pool(name="osb", bufs=3) as osb,
            tc.tile_pool(name="ps_sc", bufs=2, space="PSUM") as ps_sc,
            tc.tile_pool(name="ps_av", bufs=2, space="PSUM") as ps_av,
            tc.tile_pool(name="ps_pj", bufs=2, space="PSUM") as ps_pj,
        ):
            # exp table load warm-up: very first instruction
            warm = consts.tile([128, 8], f32, tag="warm")
            nc.vector.memset(warm, 0.0)
            nc.scalar.activation(out=warm, in_=warm, func=AF.Exp, scale=1.0)

            # const DMAs on the gpsimd queue (issue only; ring runs async).
            # xT (0,0) kt-slices ride this ring too: its preamble ends ~3us
            # before the sync ring's, so the first proj matmul starts sooner.
            xT_sb = consts.tile([128, 2, 8, S], bf16, tag="xT")
            wq_sb = consts.tile([128, 8, 128], bf16, tag="wq")
            nc.gpsimd.dma_start(out=wq_sb, in_=wq)
            for kt in range(4):
                nc.gpsimd.dma_start(out=xT_sb[:, 0, kt, 0:QCHUNK],
                                    in_=xT[0][:, kt, 0:QCHUNK])
            wk_sb = consts.tile([128, 8, 128], bf16, tag="wk")
            nc.gpsimd.dma_start(out=wk_sb, in_=wk)
            for kt in range(4, 8):
                nc.gpsimd.dma_start(out=xT_sb[:, 0, kt, 0:QCHUNK],
                                    in_=xT[0][:, kt, 0:QCHUNK])
            cos_sb = consts.tile([128, S], bf16, tag="cos")
            nc.gpsimd.dma_start(out=cos_sb, in_=cosd)
            sin_sb = consts.tile([128, S], bf16, tag="sin")
            nc.gpsimd.dma_start(out=sin_sb, in_=sind)
            lt_sb = consts.tile([128, 2, 128], bf16, tag="ltm")
            nc.gpsimd.dma_start(out=lt_sb, in_=ltd)
            wv_sb = consts.tile([128, 8, 128], bf16, tag="wv")
            nc.gpsimd.dma_start(out=wv_sb, in_=wv)
            # wo (1 MB) is not needed until outproj(0) at stage 6: load it
            # on the sync ring after the startup-critical x chunks
            wo_sb = consts.tile([128, 8, D], bf16, tag="wo")
            if with_qk_bias:
                bq_sb = consts.tile([128, 1], f32, tag="bq")
                nc.gpsimd.dma_start(out=bq_sb, in_=bqd)
                bk_sb = consts.tile([128, 1], f32, tag="bk")
                nc.gpsimd.dma_start(out=bk_sb, in_=bkd)

            # x loads on the sync queue, in first-needed order, sliced so
            # proj kt-units start as soon as their k-slice lands (the 64MB
            # aggregate flood makes every transfer slow at startup).
            for (b, c) in [(1, 0), (0, 1)]:
                cs = slice(c * QCHUNK, (c + 1) * QCHUNK)
                for kt in range(8):
                    nc.sync.dma_start(out=xT_sb[:, b, kt, cs],
                                      in_=xT[b][:, kt, cs])
            cs = slice(QCHUNK, 2 * QCHUNK)
            for kh in range(2):
                nc.sync.dma_start(out=xT_sb[:, 1, 4 * kh:4 * kh + 4, cs],
                                  in_=xT[1][:, 4 * kh:4 * kh + 4, cs])
            nc.sync.dma_start(out=wo_sb, in_=wo)
            for (b, c) in [(0, 2), (1, 2), (0, 3), (1, 3)]:
                cs = slice(c * QCHUNK, (c + 1) * QCHUNK)
                for kh in range(2):
                    nc.sync.dma_start(
                        out=xT_sb[:, b, 4 * kh:4 * kh + 4, cs],
                        in_=xT[b][:, 4 * kh:4 * kh + 4, cs])

            # Persistent rotated Q/K: [128 (=2-head pack), batch, s]
            Qr = qkpool.tile([128, 2, S], bf16, tag="Qr")
            Kr = qkpool.tile([128, 2, S], bf16, tag="Kr")
            # V with ones column: [s-part, batch, s-tile, head, 65]
            Vs = vpool.tile([128, 2, 16, 2, 65], bf16, tag="Vs")
            nc.vector.memset(Vs[:, :, :, :, 64:65], 1.0)

            # ---------------- unit helpers ----------------
            qsb_state = {}    # (w, b, c) -> q_sb tile awaiting rope
            av_state = {}     # (b, j, hh) -> z psum tile
            zsb_state = {}    # (b, j) -> normalized z sbuf tile
            zall_state = {}   # j -> zall sbuf tile

            def projqk_half(w, b, c, half):
                wsb = wq_sb if w == "q" else wk_sb
                cs = slice(c * QCHUNK, (c + 1) * QCHUNK)

                def fn():
                    if half == 0:
                        pt = ps_pj.tile([128, QCHUNK], f32, tag="pjg",
                                        bufs=1)
                        qsb_state[("pt", w, b, c)] = pt
                    else:
                        pt = qsb_state.pop(("pt", w, b, c))
                    for kt in range(4 * half, 4 * half + 4):
                        nc.tensor.matmul(
                            out=pt, lhsT=wsb[:, kt, :],
                            rhs=xT_sb[:, b, kt, cs],
                            start=(kt == 0), stop=(kt == 7),
                            skip_group_check=True)
                    if half == 1:
                        if with_qk_bias:
                            bsb = bq_sb if w == "q" else bk_sb
                            nc.vector.tensor_scalar_add(
                                out=pt, in0=pt, scalar1=bsb[:, 0:1])
                        q_sb = rope.tile([128, QCHUNK], bf16, tag="ropeA")
                        nc.vector.tensor_copy(out=q_sb, in_=pt)
                        qsb_state[(w, b, c)] = q_sb
                return fn

            def rope_unit(w, b, c):
                dst = Qr if w == "q" else Kr
                cs = slice(c * QCHUNK, (c + 1) * QCHUNK)

                def fn():
                    q_sb = qsb_state.pop((w, b, c))
                    # q_rot = q*cos + flip(q)*sin' (sign folded into sin')
                    qf = rope.tile([128, QCHUNK], bf16, tag="ropeB")
                    for blk in range(4):
                        src = (blk ^ 1) * 32
                        nc.vector.tensor_copy(
                            out=qf[blk * 32:blk * 32 + 32, :],
                            in_=q_sb[src:src + 32, :])
                    qs = rope.tile([128, QCHUNK], bf16, tag="ropeC")
                    nc.vector.tensor_tensor(
                        out=qs, in0=qf, in1=sin_sb[:, cs], op=OP.mult)
                    qc = rope.tile([128, QCHUNK], bf16, tag="ropeB")
                    nc.vector.tensor_tensor(
                        out=qc, in0=q_sb, in1=cos_sb[:, cs], op=OP.mult)
                    nc.vector.tensor_tensor(
                        out=dst[:, b, cs], in0=qc, in1=qs, op=OP.add)
                return fn

            def projv_unit(b, c, st):
                def fn():
                    pt = ps_pj.tile([128, 2, 64], f32, tag="pjm", bufs=1)
                    for kt in range(8):
                        nc.tensor.matmul(
                            out=pt,
                            lhsT=xT_sb[:, b, kt, st * 128:(st + 1) * 128],
                            rhs=wv_sb[:, kt, :],
                            start=(kt == 0), stop=(kt == 7),
                            skip_group_check=True)
                    nc.vector.tensor_copy(out=Vs[:, b, st, :, 0:64], in_=pt)
                return fn

            def av_unit(b, j, hh, p, E):
                nkt = 4 * j + 4

                def fn():
                    if p == 0:
                        z = ps_av.tile([65, 4, 128], f32, tag="av")
                        av_state[(b, j, hh)] = z
                    else:
                        z = av_state[(b, j, hh)]
                    for t in (2 * p, 2 * p + 1):
                        q0 = max(0, 128 * (t - 4 * j))
                        nc.tensor.matmul(
                            out=z[:, q0 // 128:, :], lhsT=Vs[:, b, t, hh, :],
                            rhs=E[:, t, hh, q0:],
                            start=(t == 0), stop=(t == nkt - 1),
                            skip_group_check=True)
                return fn

            def norm_unit(b, j, hh):
                def fn():
                    z = av_state.pop((b, j, hh))
                    if hh == 0:
                        zsb = zpool.tile([128, 4, 128], bf16, tag="zsb")
                        zsb_state[(b, j)] = zsb
                    else:
                        zsb = zsb_state[(b, j)]
                    hs = slice(64 * hh, 64 * hh + 64)
                    d0 = den.tile([1, 4, 128], f32, tag="d0")
                    nc.vector.tensor_copy(out=d0, in_=z[64:65, :, :])
                    nc.vector.reciprocal_approx_fast(out=d0, in_=d0)
                    rb = den.tile([64, 4, 128], f32, tag="rb")
                    nc.gpsimd.partition_broadcast(out_ap=rb, in_ap=d0)
                    nc.vector.tensor_tensor(
                        out=zsb[hs, :, :], in0=z[0:64, :, :], in1=rb,
                        op=OP.mult)
                return fn

            def send_unit(b, j):
                def fn():
                    zsb = zsb_state.pop((b, j))
                    # DRAM-side dim permutation: out iterates (p, s4, c) to
                    # match the SBUF tile's natural partition-major order
                    nc.gpsimd.dma_start(
                        out=z_send[j].ap()[4 * b:4 * b + 4].transpose(
                            [1, 0, 2]),
                        in_=zsb)
                return fn

            def trigger_unit(j):
                def fn():
                    nc.gpsimd.collective_compute(
                        "AllToAll", mybir.AluOpType.bypass,
                        replica_groups=GROUPS_ALL,
                        ins=[z_send[j].ap().opt()],
                        outs=[z_recv[j].ap().opt()])
                return fn

            def zall_unit(j):
                def fn():
                    zall = zallp.tile([128, 8, 128], bf16, tag="zall")
                    zall_state[j] = zall
                    if j < NCHUNK - 1:
                        nc.sync.dma_start(
                            out=zall,
                            in_=z_recv[j].ap().transpose([1, 0, 2]))
                    else:
                        # tail chunk: slot-pair slices so outproj's kt
                        # matmuls pipeline with the arriving data instead
                        # of waiting one big transfer's late semaphore
                        for s in range(4):
                            nc.sync.dma_start(
                                out=zall[:, 2 * s:2 * s + 2, :],
                                in_=z_recv[j].ap()[2 * s:2 * s + 2]
                                .transpose([1, 0, 2]))
                return fn

            def outproj_half(j, mc, tag="pjm", cast_scalar=False):
                def fn():
                    zall = zall_state[j]
                    po = ps_pj.tile([128, 512], f32, tag=tag, bufs=1)
                    for kt in range(8):
                        nc.tensor.matmul(
                            out=po, lhsT=zall[:, kt, :],
                            rhs=wo_sb[:, kt, mc * 512:(mc + 1) * 512],
                            start=(kt == 0), stop=(kt == 7),
                            skip_group_check=True)
                    o_sb = osb.tile([128, 512], f16, tag="osb")
                    if cast_scalar:  # tail: ScalarE idle, Vector backed up
                        nc.scalar.activation(out=o_sb, in_=po, func=AF.Copy)
                    else:
                        nc.vector.tensor_copy(out=o_sb, in_=po)
                    nc.sync.dma_start(
                        out=out_ext[j * 128:(j + 1) * 128,
                                    mc * 512:(mc + 1) * 512],
                        in_=o_sb)
                return fn

            def proj_units(b, c):
                # interleave the pjg users (qk halves) with pjm users
                # (V s-tiles) so each pool's WAR-on-cast latency is hidden.
                # key=(b,c) marks units that stage (b,c)'s scores depend on.
                sts = list(range(4 * c, 4 * c + 4))
                us = [
                    (0, 1.05, projqk_half("q", b, c, 0), (b, c)),
                    (0, 1.05, projqk_half("q", b, c, 1), (b, c)),
                    (0, 0.05, rope_unit("q", b, c), (b, c)),
                    (0, 0.55, projv_unit(b, c, sts[0]), (b, c)),
                    (0, 1.05, projqk_half("k", b, c, 0), (b, c)),
                    (0, 0.55, projv_unit(b, c, sts[1]), (b, c)),
                    (0, 1.05, projqk_half("k", b, c, 1), (b, c)),
                    (0, 0.05, rope_unit("k", b, c), (b, c)),
                    (0, 0.55, projv_unit(b, c, sts[2]), (b, c)),
                    (0, 0.55, projv_unit(b, c, sts[3]), (b, c)),
                ]
                return us

            # ---------------- stage driver ----------------
            units = []  # (gate_tile, cost_us, fn, key)

            def drain(t, budget):
                while True:
                    idx = None
                    for i, (g, cst, fn, key) in enumerate(units):
                        if g > t:
                            continue  # gated: may be jumped (independent)
                        if cst <= budget + 1.2:
                            idx = i
                        break  # first READY unit pops or blocks the drain
                    if idx is None:
                        return budget
                    g, cst, fn, key = units.pop(idx)
                    fn()
                    budget -= cst

            def run_stage(b, j):
                nkt = 4 * j + 4
                # force-drain any projection units this stage's scores
                # depend on (Qr/Kr/Vs writers for (b, <=j)) — their writes
                # must be ISSUED before the first score matmul reads them
                last = max((i for i, u in enumerate(units)
                            if u[3] is not None and u[3][0] == b
                            and u[3][1] <= j), default=-1)
                for _ in range(last + 1):
                    g, cst, fn, key = units.pop(0)
                    fn()
                # units carried over from earlier stages have stale gates;
                # all their deps are already issued, so make them ready now
                units[:] = [(0, cst, fn, key)
                            for (_, cst, fn, key) in units]
                E = epool.tile([128, 16, 2, QCHUNK], bf16, tag="E")
                # this stage's AV + normalize + send, gated on exp progress
                for p in range(nkt // 2):
                    units.append((2 * p + 4, 0.55,
                                  av_unit(b, j, 0, p, E), None))
                    units.append((2 * p + 5, 0.55,
                                  av_unit(b, j, 1, p, E), None))
                units.append((nkt, 0.05, norm_unit(b, j, 0), None))
                units.append((nkt, 0.05, norm_unit(b, j, 1), None))
                units.append((nkt, 0.05, send_unit(b, j), None))
                # trigger right after the batch-1 send (chunk 0 deferred:
                # its sends crawl behind the startup HBM flood)
                if b == 1 and j >= 1:
                    units.append((nkt, 0.05, trigger_unit(j), None))

                budget = 0.0
                for t in range(nkt):
                    q0 = max(0, 128 * (t - 4 * j))
                    qs2 = slice(j * QCHUNK + q0, (j + 1) * QCHUNK)
                    sc = ps_sc.tile([128, 2, QCHUNK], f32, tag="sc")
                    for hh in range(2):
                        hs = slice(64 * hh, 64 * hh + 64)
                        nc.tensor.matmul(
                            out=sc[:, hh, q0:],
                            lhsT=Kr[hs, b, t * 128:(t + 1) * 128],
                            rhs=Qr[hs, b, qs2], start=True, stop=True)
                    nc.scalar.activation(
                        out=E[:, t, :, q0:], in_=sc[:, :, q0:],
                        func=AF.Exp, scale=1.0 / SCALE)
                    if t >= 4 * j:  # diagonal tile: causal mask (q >= k)
                        qb = slice(q0, q0 + 128)
                        nc.vector.tensor_tensor(
                            out=E[:, t, :, qb], in0=E[:, t, :, qb],
                            in1=lt_sb, op=OP.mult)
                    budget += 0.55 * (QCHUNK - q0) / QCHUNK
                    budget = drain(t, budget)
                # leftovers carry into the next stage's burst, so the PE
                # never sits through a serial stage-end drain

            # ---------------- schedule ----------------
            # stage 0 (b=0, j=0): its own projections run inline first
            for g, cst, fn, key in proj_units(0, 0):
                fn()
            stages = [(b, j) for j in range(NCHUNK) for b in range(2)]
            for idx, (b, j) in enumerate(stages):
                # chunk 0's trigger one stage after its sends, so the
                # gpsimd queue never blocks on flood-era send DMAs
                if idx == 2:
                    units.append((4, 0.05, trigger_unit(0), None))
                # outproj(0)/(1) in stage 7 only: the early AllToAlls crawl
                # behind the 64MB x-load flood plus inter-core skew, and a
                # premature outproj matmul blocks the in-order PE queue
                if idx == 7:
                    units.append((1, 0.10, zall_unit(0), None))
                    units.append((2, 1.15, outproj_half(0, 0), None))
                    units.append((3, 1.15, outproj_half(0, 1), None))
                    units.append((4, 0.10, zall_unit(1), None))
                    units.append((5, 1.15, outproj_half(1, 0), None))
                    units.append((6, 1.15, outproj_half(1, 1), None))
                # projections for upcoming chunks
                if idx == 0:
                    units.extend(proj_units(1, 0))
                    units.extend(proj_units(0, 1))
                elif j < NCHUNK - 1:
                    units.extend(proj_units(b, j + 1))
                run_stage(b, j)
            while units:  # final stage's AV tail, norms, sends
                g, cst, fn, key = units.pop(0)
                fn()

            # epilogue: cover the final AllToAll (triggered in the last
            # stage's drain) with outproj(NCHUNK-2), then the last chunk.
            for jp in range(NCHUNK - 2, NCHUNK):
                zall_unit(jp)()
                outproj_half(jp, 0, tag="pjg", cast_scalar=True)()
                outproj_half(jp, 1, cast_scalar=True)()

    nc.compile()
    return nc


def _get_built(with_qk_bias):
    key = bool(with_qk_bias)
    if key not in _BUILT:
        _BUILT[key] = _build(key)
    return _BUILT[key]


def _rope_tables():
    pos = np.arange(S, dtype=np.float64)
    dim = np.arange(DH // 2, dtype=np.float64)
    freq = ROT_BASE ** (dim / (DH / 2))
    freq = np.concatenate([freq, freq])                # [64]
    ang = pos[None, :] / freq[:, None]                 # [64, S]
    cos = np.cos(ang)
    sin = np.sin(ang)
    # sign of the rotate-half term folded into sin': rows 0..31 get -sin
    sinm = sin.copy()
    sinm[:DH // 2] *= -1.0
    cosT = np.tile(cos, (2, 1)).astype(BF)             # [128, S]
    sinT = np.tile(sinm, (2, 1)).astype(BF)
    return cosT, sinT


def kernel(x, W_Q, b_Q, W_K, b_K, W_V, b_V, W_O, b_O):
    from concourse.bass_utils import run_bass_kernel_spmd

    x = np.asarray(x)
    W_Q, W_K, W_V, W_O = (np.asarray(a) for a in (W_Q, W_K, W_V, W_O))
    b_Q, b_K, b_V, b_O = (np.asarray(a) for a in (b_Q, b_K, b_V, b_O))
    with_qk_bias = bool(np.any(b_Q) or np.any(b_K))
    nc = _get_built(with_qk_bias)

    cosT, sinT = _rope_tables()
    lt = np.tril(np.ones((128, 128), dtype=np.float32)).T  # [k, q]: q >= k
    ltm = np.ascontiguousarray(
        np.broadcast_to(lt[:, None, :], (128, 2, 128))).astype(BF)

    def wtile(w):            # [1024, C] -> [128, 8, C]
        c = w.shape[1]
        return np.ascontiguousarray(
            w.reshape(8, 128, c).transpose(1, 0, 2)).astype(BF)

    # x transposed per batch: [d, s]: d = kt*128 + p -> [p, kt, s]
    xT_host = np.stack([
        np.ascontiguousarray(
            x[b].T.reshape(8, 128, S).transpose(1, 0, 2)).astype(BF)
        for b in range(2)], axis=0)
    # W_O for ALL heads: slot s = heads (2s, 2s+1); identical on all cores
    wo_h = np.ascontiguousarray(
        np.concatenate([W_O[h] for h in range(NH)], axis=0)  # [1024, 1024]
        .reshape(8, 128, D).transpose(1, 0, 2)).astype(BF)

    in_maps = []
    for core in range(N_CORES):
        h0 = 2 * core
        wq_h = wtile(np.concatenate([W_Q[h0], W_Q[h0 + 1]], axis=1))
        wk_h = wtile(np.concatenate([W_K[h0], W_K[h0 + 1]], axis=1))
        wv_h = wtile(np.concatenate([W_V[h0], W_V[h0 + 1]], axis=1))
        m = {
            "xT": xT_host, "wq": wq_h, "wk": wk_h, "wv": wv_h, "wo": wo_h,
            "cosT": cosT, "sinTm": sinT, "ltm": ltm,
        }
        if with_qk_bias:
            m["bq"] = np.concatenate(
                [b_Q[h0], b_Q[h0 + 1]]).astype(np.float32)[:, None]
            m["bk"] = np.concatenate(
                [b_K[h0], b_K[h0 + 1]]).astype(np.float32)[:, None]
        in_maps.append(m)

    global _last_in_maps
    _last_in_maps = in_maps
    res = run_bass_kernel_spmd(nc, in_maps, list(range(N_CORES)))

    out = np.empty((2, S, D), dtype=np.float32)
    for core in range(N_CORES):
        b, r = divmod(core, 4)
        shard = res.results[core]["out_shard"].astype(np.float32)
        for j in range(NCHUNK):
            out[b, QCHUNK * j + 128 * r: QCHUNK * j + 128 * (r + 1), :] = \
                shard[128 * j:128 * (j + 1)]

    # b_V shifts z by exactly b_V (softmax rows sum to 1); fold with b_O.
    corr = b_O.astype(np.float64).copy()
    if np.any(b_V):
        corr = corr + np.einsum("hd,hdm->m", b_V.astype(np.float64),
                                W_O.astype(np.float64))
    if np.any(corr):
        out = out + corr.astype(np.float32)
    return out


# revision 75
# speedup vs baseline: 1.5178x; 1.0293x over previous
"""Trainium2 Bass kernel for causal multi-head attention with NeoX RoPE.

Problem: x[2, 2048, 1024], 16 heads x d_head 64, rotary over all 64 dims,
causal softmax, output projection.

Sharding: every core holds 2 heads ({2c, 2c+1}) and processes BOTH
batches.  After a per-q-chunk 8-core AllToAll of the normalized z shards,
each core contracts all 16 heads locally and writes its own [128 x 1024]
output rows per chunk.

v3 scheduling: all work is decomposed into ~0.5-1.1us units (proj kt
halves, V-proj per s-tile, AV 2-ktile groups, outproj halves, rope,
normalize, DMA) drained between score tiles with a cost budget, so the
PE never idles while exp paces the score psum recycle.  AV of a stage
runs inside the SAME stage's burst (gated on exp progress), which keeps
the tail short.  Collective-chain DMAs (z_send/zall/out) live on the
Sync queue; GpSimd only does affine_select + partition_broadcast, so the
collective wait never head-of-line-blocks the causal mask.  The final
chunk's outproj(2) is reserved to overlap the last AllToAll.
"""

import numpy as np
import ml_dtypes

S = 2048
D = 1024
NH = 16
DH = 64
SCALE = 8.0
ROT_BASE = 10000.0
N_CORES = 8
QCHUNK = 512     # q chunk (free dim of score matmuls)
NCHUNK = S // QCHUNK
KTILE = 128
BF = ml_dtypes.bfloat16
GROUPS_ALL = [[0, 1, 2, 3, 4, 5, 6, 7]]

_BUILT = {}


def _build(with_qk_bias):
    import concourse.bass as bass
    import concourse.tile as tile
    from concourse import bacc, mybir

    f32 = mybir.dt.float32
    bf16 = mybir.dt.bfloat16
    f16 = mybir.dt.float16
    AF = mybir.ActivationFunctionType
    OP = mybir.AluOpType

    nc = bacc.Bacc("TRN2", target_bir_lowering=False, debug=False,
                   num_devices=N_CORES)

    xT = nc.dram_tensor("xT", [2, 128, 8, S], bf16, kind="ExternalInput").ap()
    wq = nc.dram_tensor("wq", [128, 8, 128], bf16, kind="ExternalInput").ap()
    wk = nc.dram_tensor("wk", [128, 8, 128], bf16, kind="ExternalInput").ap()
    wv = nc.dram_tensor("wv", [128, 8, 128], bf16, kind="ExternalInput").ap()
    wo = nc.dram_tensor("wo", [128, 8, D], bf16, kind="ExternalInput").ap()
    cosd = nc.dram_tensor("cosT", [128, S], bf16, kind="ExternalInput").ap()
    sind = nc.dram_tensor("sinTm", [128, S], bf16, kind="ExternalInput").ap()
    ltd = nc.dram_tensor("ltm", [128, 2, 128], bf16, kind="ExternalInput").ap()
    if with_qk_bias:
        bqd = nc.dram_tensor("bq", [128, 1], f32, kind="ExternalInput").ap()
        bkd = nc.dram_tensor("bk", [128, 1], f32, kind="ExternalInput").ap()

    z_send = [nc.dram_tensor(f"z_send{j}", [8, 128, 128], bf16)
              for j in range(NCHUNK)]
    z_recv = [nc.dram_tensor(f"z_recv{j}", [8, 128, 128], bf16)
              for j in range(NCHUNK)]
    # chunks 0+1 exchanged together: 3 collectives instead of 4 keeps the
    # serialized CC channel off its saturation point
    z_send01 = nc.dram_tensor("z_send01", [8, 128, 256], bf16)
    z_recv01 = nc.dram_tensor("z_recv01", [8, 128, 256], bf16)
    out_ext = nc.dram_tensor("out_shard", [S // 4, D], f16,
                             kind="ExternalOutput").ap()

    with tile.TileContext(nc) as tc:
        with (
            tc.tile_pool(name="consts", bufs=1) as consts,
            tc.tile_pool(name="qk", bufs=1) as qkpool,
            tc.tile_pool(name="vsb", bufs=1) as vpool,
            tc.tile_pool(name="rope", bufs=2) as rope,
            tc.tile_pool(name="epool", bufs=2) as epool,
            tc.tile_pool(name="zpool", bufs=4) as zpool,
            tc.tile_pool(name="den", bufs=2) as den,
            tc.tile_pool(name="zail", bufs=2) as zallp,
            tc.tile_pool(name="osb", bufs=3) as osb,
            tc.tile_pool(name="ps_sc", bufs=2, space="PSUM") as ps_sc,
            tc.tile_pool(name="ps_av", bufs=2, space="PSUM") as ps_av,
            tc.tile_pool(name="ps_pj", bufs=2, space="PSUM") as ps_pj,
        ):
            # exp table load warm-up: very first instruction
            warm = consts.tile([128, 8], f32, tag="warm")
            nc.vector.memset(warm, 0.0)
            nc.scalar.activation(out=warm, in_=warm, func=AF.Exp, scale=1.0)

            # const DMAs on the gpsimd queue (issue only; ring runs async).
            # xT (0,0) kt-slices ride this ring too: its preamble ends ~3us
            # before the sync ring's, so the first proj matmul starts sooner.
            xT_sb = consts.tile([128, 2, 8, S], bf16, tag="xT")
            wq_sb = consts.tile([128, 8, 128], bf16, tag="wq")
            nc.gpsimd.dma_start(out=wq_sb, in_=wq)
            for kt in range(4):
                nc.gpsimd.dma_start(out=xT_sb[:, 0, kt, 0:QCHUNK],
                                    in_=xT[0][:, kt, 0:QCHUNK])
            wk_sb = consts.tile([128, 8, 128], bf16, tag="wk")
            nc.gpsimd.dma_start(out=wk_sb, in_=wk)
            for kt in range(4, 8):
                nc.gpsimd.dma_start(out=xT_sb[:, 0, kt, 0:QCHUNK],
                                    in_=xT[0][:, kt, 0:QCHUNK])
            cos_sb = consts.tile([128, S], bf16, tag="cos")
            nc.gpsimd.dma_start(out=cos_sb, in_=cosd)
            sin_sb = consts.tile([128, S], bf16, tag="sin")
            nc.gpsimd.dma_start(out=sin_sb, in_=sind)
            lt_sb = consts.tile([128, 2, 128], bf16, tag="ltm")
            nc.gpsimd.dma_start(out=lt_sb, in_=ltd)
            wv_sb = consts.tile([128, 8, 128], bf16, tag="wv")
            nc.gpsimd.dma_start(out=wv_sb, in_=wv)
            # wo (1 MB) is not needed until outproj(0) at stage 6: load it
            # on the sync ring after the startup-critical x chunks
            wo_sb = consts.tile([128, 8, D], bf16, tag="wo")
            if with_qk_bias:
                bq_sb = consts.tile([128, 1], f32, tag="bq")
                nc.gpsimd.dma_start(out=bq_sb, in_=bqd)
                bk_sb = consts.tile([128, 1], f32, tag="bk")
                nc.gpsimd.dma_start(out=bk_sb, in_=bkd)

            # x loads on the sync queue, in first-needed order, sliced so
            # proj kt-units start as soon as their k-slice lands (the 64MB
            # aggregate flood makes every transfer slow at startup).
            for (b, c) in [(1, 0), (0, 1)]:
                cs = slice(c * QCHUNK, (c + 1) * QCHUNK)
                for kt in range(8):
                    nc.sync.dma_start(out=xT_sb[:, b, kt, cs],
                                      in_=xT[b][:, kt, cs])
            cs = slice(QCHUNK, 2 * QCHUNK)
            for kh in range(2):
                nc.sync.dma_start(out=xT_sb[:, 1, 4 * kh:4 * kh + 4, cs],
                                  in_=xT[1][:, 4 * kh:4 * kh + 4, cs])
            nc.sync.dma_start(out=wo_sb, in_=wo)
            for (b, c) in [(0, 2), (1, 2), (0, 3), (1, 3)]:
                cs = slice(c * QCHUNK, (c + 1) * QCHUNK)
                for kh in range(2):
                    nc.sync.dma_start(
                        out=xT_sb[:, b, 4 * kh:4 * kh + 4, cs],
                        in_=xT[b][:, 4 * kh:4 * kh + 4, cs])

            # Persistent rotated Q/K: [128 (=2-head pack), batch, s]
            Qr = qkpool.tile([128, 2, S], bf16, tag="Qr")
            Kr = qkpool.tile([128, 2, S], bf16, tag="Kr")
            # V with ones column: [s-part, batch, s-tile, head, 65]
            Vs = vpool.tile([128, 2, 16, 2, 65], bf16, tag="Vs")
            nc.vector.memset(Vs[:, :, :, :, 64:65], 1.0)

            # ---------------- unit helpers ----------------
            qsb_state = {}    # (w, b, c) -> q_sb tile awaiting rope
            av_state = {}     # (b, j, hh) -> z psum tile
            zsb_state = {}    # (b, j) -> normalized z sbuf tile
            zall_state = {}   # j -> zall sbuf tile

            def projqk_half(w, b, c, half):
                wsb = wq_sb if w == "q" else wk_sb
                cs = slice(c * QCHUNK, (c + 1) * QCHUNK)

                def fn():
                    if half == 0:
                        pt = ps_pj.tile([128, QCHUNK], f32, tag="pjg",
                                        bufs=1)
                        qsb_state[("pt", w, b, c)] = pt
                    else:
                        pt = qsb_state.pop(("pt", w, b, c))
                    for kt in range(4 * half, 4 * half + 4):
                        nc.tensor.matmul(
                            out=pt, lhsT=wsb[:, kt, :],
                            rhs=xT_sb[:, b, kt, cs],
                            start=(kt == 0), stop=(kt == 7),
                            skip_group_check=True)
                    if half == 1:
                        if with_qk_bias:
                            bsb = bq_sb if w == "q" else bk_sb
                            nc.vector.tensor_scalar_add(
                                out=pt, in0=pt, scalar1=bsb[:, 0:1])
                        q_sb = rope.tile([128, QCHUNK], bf16, tag="ropeA")
                        nc.vector.tensor_copy(out=q_sb, in_=pt)
                        qsb_state[(w, b, c)] = q_sb
                return fn

            def rope_unit(w, b, c):
                dst = Qr if w == "q" else Kr
                cs = slice(c * QCHUNK, (c + 1) * QCHUNK)

                def fn():
                    q_sb = qsb_state.pop((w, b, c))
                    # q_rot = q*cos + flip(q)*sin' (sign folded into sin')
                    qf = rope.tile([128, QCHUNK], bf16, tag="ropeB")
                    for blk in range(4):
                        src = (blk ^ 1) * 32
                        nc.vector.tensor_copy(
                            out=qf[blk * 32:blk * 32 + 32, :],
                            in_=q_sb[src:src + 32, :])
                    qs = rope.tile([128, QCHUNK], bf16, tag="ropeC")
                    nc.vector.tensor_tensor(
                        out=qs, in0=qf, in1=sin_sb[:, cs], op=OP.mult)
                    qc = rope.tile([128, QCHUNK], bf16, tag="ropeB")
                    nc.vector.tensor_tensor(
                        out=qc, in0=q_sb, in1=cos_sb[:, cs], op=OP.mult)
                    nc.vector.tensor_tensor(
                        out=dst[:, b, cs], in0=qc, in1=qs, op=OP.add)
                return fn

            def projv_unit(b, c, st):
                def fn():
                    pt = ps_pj.tile([128, 2, 64], f32, tag="pjm", bufs=1)
                    for kt in range(8):
                        nc.tensor.matmul(
                            out=pt,
                            lhsT=xT_sb[:, b, kt, st * 128:(st + 1) * 128],
                            rhs=wv_sb[:, kt, :],
                            start=(kt == 0), stop=(kt == 7),
                            skip_group_check=True)
                    nc.vector.tensor_copy(out=Vs[:, b, st, :, 0:64], in_=pt)
                return fn

            def av_unit(b, j, hh, p, E):
                nkt = 4 * j + 4

                def fn():
                    if p == 0:
                        z = ps_av.tile([65, 4, 128], f32, tag="av")
                        av_state[(b, j, hh)] = z
                    else:
                        z = av_state[(b, j, hh)]
                    for t in (2 * p, 2 * p + 1):
                        q0 = max(0, 128 * (t - 4 * j))
                        nc.tensor.matmul(
                            out=z[:, q0 // 128:, :], lhsT=Vs[:, b, t, hh, :],
                            rhs=E[:, t, hh, q0:],
                            start=(t == 0), stop=(t == nkt - 1),
                            skip_group_check=True)
                return fn

            def norm_unit(b, j, hh):
                def fn():
                    z = av_state.pop((b, j, hh))
                    if hh == 0:
                        zsb = zpool.tile([128, 4, 128], bf16, tag="zsb")
                        zsb_state[(b, j)] = zsb
                    else:
                        zsb = zsb_state[(b, j)]
                    hs = slice(64 * hh, 64 * hh + 64)
                    d0 = den.tile([1, 4, 128], f32, tag="d0")
                    nc.vector.tensor_copy(out=d0, in_=z[64:65, :, :])
                    nc.vector.reciprocal_approx_fast(out=d0, in_=d0)
                    rb = den.tile([64, 4, 128], f32, tag="rb")
                    nc.gpsimd.partition_broadcast(out_ap=rb, in_ap=d0)
                    nc.vector.tensor_tensor(
                        out=zsb[hs, :, :], in0=z[0:64, :, :], in1=rb,
                        op=OP.mult)
                return fn

            def send_unit(b, j):
                def fn():
                    zsb = zsb_state.pop((b, j))
                    # DRAM-side dim permutation: out iterates (p, s4, c) to
                    # match the SBUF tile's natural partition-major order
                    if j < 2:
                        out_ap = z_send01.ap()[4 * b:4 * b + 4][
                            :, :, 128 * j:128 * j + 128]
                    else:
                        out_ap = z_send[j].ap()[4 * b:4 * b + 4]
                    nc.gpsimd.dma_start(out=out_ap.transpose([1, 0, 2]),
                                        in_=zsb)
                return fn

            def trigger_unit(j):
                def fn():
                    if j < 2:  # merged chunks 0+1
                        nc.gpsimd.collective_compute(
                            "AllToAll", mybir.AluOpType.bypass,
                            replica_groups=GROUPS_ALL,
                            ins=[z_send01.ap().opt()],
                            outs=[z_recv01.ap().opt()])
                    else:
                        nc.gpsimd.collective_compute(
                            "AllToAll", mybir.AluOpType.bypass,
                            replica_groups=GROUPS_ALL,
                            ins=[z_send[j].ap().opt()],
                            outs=[z_recv[j].ap().opt()])
                return fn

            def zall_unit(j):
                def fn():
                    zall = zallp.tile([128, 8, 128], bf16, tag="zall")
                    zall_state[j] = zall
                    if j < 2:
                        nc.sync.dma_start(
                            out=zall,
                            in_=z_recv01.ap()[:, :, 128 * j:128 * j + 128]
                            .transpose([1, 0, 2]))
                    elif j < NCHUNK - 1:
                        nc.sync.dma_start(
                            out=zall,
                            in_=z_recv[j].ap().transpose([1, 0, 2]))
                    else:
                        # tail chunk: slot-pair slices so outproj's kt
                        # matmuls pipeline with the arriving data instead
                        # of waiting one big transfer's late semaphore
                        for s in range(4):
                            nc.sync.dma_start(
                                out=zall[:, 2 * s:2 * s + 2, :],
                                in_=z_recv[j].ap()[2 * s:2 * s + 2]
                                .transpose([1, 0, 2]))
                return fn

            def outproj_half(j, mc, tag="pjm", cast_scalar=False):
                def fn():
                    zall = zall_state[j]
                    po = ps_pj.tile([128, 512], f32, tag=tag, bufs=1)
                    for kt in range(8):
                        nc.tensor.matmul(
                            out=po, lhsT=zall[:, kt, :],
                            rhs=wo_sb[:, kt, mc * 512:(mc + 1) * 512],
                            start=(kt == 0), stop=(kt == 7),
                            skip_group_check=True)
                    o_sb = osb.tile([128, 512], f16, tag="osb")
                    if cast_scalar:  # tail: ScalarE idle, Vector backed up
                        nc.scalar.activation(out=o_sb, in_=po, func=AF.Copy)
                    else:
                        nc.vector.tensor_copy(out=o_sb, in_=po)
                    nc.sync.dma_start(
                        out=out_ext[j * 128:(j + 1) * 128,
                                    mc * 512:(mc + 1) * 512],
                        in_=o_sb)
                return fn

            def proj_units(b, c):
                # interleave the pjg users (qk halves) with pjm users
                # (V s-tiles) so each pool's WAR-on-cast latency is hidden.
                # key=(b,c) marks units that stage (b,c)'s scores depend on.
                sts = list(range(4 * c, 4 * c + 4))
                us = [
                    (0, 1.05, projqk_half("q", b, c, 0), (b, c)),
                    (0, 1.05, projqk_half("q", b, c, 1), (b, c)),
                    (0, 0.05, rope_unit("q", b, c), (b, c)),
                    (0, 0.55, projv_unit(b, c, sts[0]), (b, c)),
                    (0, 1.05, projqk_half("k", b, c, 0), (b, c)),
                    (0, 0.55, projv_unit(b, c, sts[1]), (b, c)),
                    (0, 1.05, projqk_half("k", b, c, 1), (b, c)),
                    (0, 0.05, rope_unit("k", b, c), (b, c)),
                    (0, 0.55, projv_unit(b, c, sts[2]), (b, c)),
                    (0, 0.55, projv_unit(b, c, sts[3]), (b, c)),
                ]
                return us

            # ---------------- stage driver ----------------
            units = []  # (gate_tile, cost_us, fn, key)

            def drain(t, budget):
                while True:
                    idx = None
                    for i, (g, cst, fn, key) in enumerate(units):
                        if g > t:
                            continue  # gated: may be jumped (independent)
                        if cst <= budget + 1.2:
                            idx = i
                        break  # first READY unit pops or blocks the drain
                    if idx is None:
                        return budget
                    g, cst, fn, key = units.pop(idx)
                    fn()
                    budget -= cst

            def run_stage(b, j):
                nkt = 4 * j + 4
                # force-drain any projection units this stage's scores
                # depend on (Qr/Kr/Vs writers for (b, <=j)) — their writes
                # must be ISSUED before the first score matmul reads them
                last = max((i for i, u in enumerate(units)
                            if u[3] is not None and u[3][0] == b
                            and u[3][1] <= j), default=-1)
                for _ in range(last + 1):
                    g, cst, fn, key = units.pop(0)
                    fn()
                # units carried over from earlier stages have stale gates;
                # all their deps are already issued, so make them ready now
                units[:] = [(0, cst, fn, key)
                            for (_, cst, fn, key) in units]
                E = epool.tile([128, 16, 2, QCHUNK], bf16, tag="E")
                # this stage's AV + normalize + send, gated on exp progress
                for p in range(nkt // 2):
                    units.append((2 * p + 4, 0.55,
                                  av_unit(b, j, 0, p, E), None))
                    units.append((2 * p + 5, 0.55,
                                  av_unit(b, j, 1, p, E), None))
                units.append((nkt, 0.05, norm_unit(b, j, 0), None))
                units.append((nkt, 0.05, norm_unit(b, j, 1), None))
                units.append((nkt, 0.05, send_unit(b, j), None))
                # trigger right after the batch-1 send; chunks 0+1 fire as
                # one merged collective once all four of their sends landed
                if b == 1 and j >= 1:
                    units.append((nkt, 0.05, trigger_unit(j), None))

                budget = 0.0
                for t in range(nkt):
                    q0 = max(0, 128 * (t - 4 * j))
                    qs2 = slice(j * QCHUNK + q0, (j + 1) * QCHUNK)
                    sc = ps_sc.tile([128, 2, QCHUNK], f32, tag="sc")
                    for hh in range(2):
                        hs = slice(64 * hh, 64 * hh + 64)
                        nc.tensor.matmul(
                            out=sc[:, hh, q0:],
                            lhsT=Kr[hs, b, t * 128:(t + 1) * 128],
                            rhs=Qr[hs, b, qs2], start=True, stop=True)
                    nc.scalar.activation(
                        out=E[:, t, :, q0:], in_=sc[:, :, q0:],
                        func=AF.Exp, scale=1.0 / SCALE)
                    if t >= 4 * j:  # diagonal tile: causal mask (q >= k)
                        qb = slice(q0, q0 + 128)
                        nc.vector.tensor_tensor(
                            out=E[:, t, :, qb], in0=E[:, t, :, qb],
                            in1=lt_sb, op=OP.mult)
                    budget += 0.55 * (QCHUNK - q0) / QCHUNK
                    budget = drain(t, budget)
                # leftovers carry into the next stage's burst, so the PE
                # never sits through a serial stage-end drain

            # ---------------- schedule ----------------
            # stage 0 (b=0, j=0): its own projections run inline first
            for g, cst, fn, key in proj_units(0, 0):
                fn()
            stages = [(b, j) for j in range(NCHUNK) for b in range(2)]
            for idx, (b, j) in enumerate(stages):

                # outproj(0)/(1) in stage 7 only: the early AllToAlls crawl
                # behind the 64MB x-load flood plus inter-core skew, and a
                # premature outproj matmul blocks the in-order PE queue
                if idx == 7:
                    units.append((1, 0.10, zall_unit(0), None))
                    units.append((2, 1.15, outproj_half(0, 0), None))
                    units.append((3, 1.15, outproj_half(0, 1), None))
                    units.append((4, 0.10, zall_unit(1), None))
                    units.append((5, 1.15, outproj_half(1, 0), None))
                    units.append((6, 1.15, outproj_half(1, 1), None))
                # projections for upcoming chunks
                if idx == 0:
                    units.extend(proj_units(1, 0))
                    units.extend(proj_units(0, 1))
                elif j < NCHUNK - 1:
                    units.extend(proj_units(b, j + 1))
                run_stage(b, j)
            while units:  # final stage's AV tail, norms, sends
                g, cst, fn, key = units.pop(0)
                fn()

            # epilogue: cover the final AllToAll (triggered in the last
            # stage's drain) with outproj(NCHUNK-2), then the last chunk.
            for jp in range(NCHUNK - 2, NCHUNK):
                zall_unit(jp)()
                outproj_half(jp, 0, tag="pjg", cast_scalar=True)()
                outproj_half(jp, 1, cast_scalar=True)()

    nc.compile()
    return nc


def _get_built(with_qk_bias):
    key = bool(with_qk_bias)
    if key not in _BUILT:
        _BUILT[key] = _build(key)
    return _BUILT[key]


def _rope_tables():
    pos = np.arange(S, dtype=np.float64)
    dim = np.arange(DH // 2, dtype=np.float64)
    freq = ROT_BASE ** (dim / (DH / 2))
    freq = np.concatenate([freq, freq])                # [64]
    ang = pos[None, :] / freq[:, None]                 # [64, S]
    cos = np.cos(ang)
    sin = np.sin(ang)
    # sign of the rotate-half term folded into sin': rows 0..31 get -sin
    sinm = sin.copy()
    sinm[:DH // 2] *= -1.0
    cosT = np.tile(cos, (2, 1)).astype(BF)             # [128, S]
    sinT = np.tile(sinm, (2, 1)).astype(BF)
    return cosT, sinT


def kernel(x, W_Q, b_Q, W_K, b_K, W_V, b_V, W_O, b_O):
    from concourse.bass_utils import run_bass_kernel_spmd

    x = np.asarray(x)
    W_Q, W_K, W_V, W_O = (np.asarray(a) for a in (W_Q, W_K, W_V, W_O))
    b_Q, b_K, b_V, b_O = (np.asarray(a) for a in (b_Q, b_K, b_V, b_O))
    with_qk_bias = bool(np.any(b_Q) or np.any(b_K))
    nc = _get_built(with_qk_bias)

    cosT, sinT = _rope_tables()
    lt = np.tril(np.ones((128, 128), dtype=np.float32)).T  # [k, q]: q >= k
    ltm = np.ascontiguousarray(
        np.broadcast_to(lt[:, None, :], (128, 2, 128))).astype(BF)

    def wtile(w):            # [1024, C] -> [128, 8, C]
        c = w.shape[1]
        return np.ascontiguousarray(
            w.reshape(8, 128, c).transpose(1, 0, 2)).astype(BF)

    # x transposed per batch: [d, s]: d = kt*128 + p -> [p, kt, s]
    xT_host = np.stack([
        np.ascontiguousarray(
            x[b].T.reshape(8, 128, S).transpose(1, 0, 2)).astype(BF)
        for b in range(2)], axis=0)
    # W_O for ALL heads: slot s = heads (2s, 2s+1); identical on all cores
    wo_h = np.ascontiguousarray(
        np.concatenate([W_O[h] for h in range(NH)], axis=0)  # [1024, 1024]
        .reshape(8, 128, D).transpose(1, 0, 2)).astype(BF)

    in_maps = []
    for core in range(N_CORES):
        h0 = 2 * core
        wq_h = wtile(np.concatenate([W_Q[h0], W_Q[h0 + 1]], axis=1))
        wk_h = wtile(np.concatenate([W_K[h0], W_K[h0 + 1]], axis=1))
        wv_h = wtile(np.concatenate([W_V[h0], W_V[h0 + 1]], axis=1))
        m = {
            "xT": xT_host, "wq": wq_h, "wk": wk_h, "wv": wv_h, "wo": wo_h,
            "cosT": cosT, "sinTm": sinT, "ltm": ltm,
        }
        if with_qk_bias:
            m["bq"] = np.concatenate(
                [b_Q[h0], b_Q[h0 + 1]]).astype(np.float32)[:, None]
            m["bk"] = np.concatenate(
                [b_K[h0], b_K[h0 + 1]]).astype(np.float32)[:, None]
        in_maps.append(m)

    global _last_in_maps
    _last_in_maps = in_maps
    res = run_bass_kernel_spmd(nc, in_maps, list(range(N_CORES)))

    out = np.empty((2, S, D), dtype=np.float32)
    for core in range(N_CORES):
        b, r = divmod(core, 4)
        shard = res.results[core]["out_shard"].astype(np.float32)
        for j in range(NCHUNK):
            out[b, QCHUNK * j + 128 * r: QCHUNK * j + 128 * (r + 1), :] = \
                shard[128 * j:128 * (j + 1)]

    # b_V shifts z by exactly b_V (softmax rows sum to 1); fold with b_O.
    corr = b_O.astype(np.float64).copy()
    if np.any(b_V):
        corr = corr + np.einsum("hd,hdm->m", b_V.astype(np.float64),
                                W_O.astype(np.float64))
    if np.any(corr):
        out = out + corr.astype(np.float32)
    return out
